# revision 34
# baseline (speedup 1.0000x reference)
"""Distributed Trainium2 kernel for a transformer attention block (B=2, S=4096,
D=1024, H=4096, fp32 I/O).

Reference computation (note the Q<-k, K<-q, V<-v argument quirk):
    k = x @ Wk + bk ; q = x @ Wq + bq ; v = x @ Wv + bv
    scores[s,t] = k[s]·q[t] / sqrt(D); attn = softmax_t(scores) @ v
    x1 = LN(x + attn); h = gelu(x1 @ W1 + b1); out = LN(x1 + h @ W2 + b2)

Sharding: 8 cores -> 2 groups of 4 (one group per batch element); each core
owns 1024 sequence rows. Design notes:
  - all five GEMMs run fp8 (e4m3) DoubleRow matmuls. Weights are pre-cast/
    pre-tiled on the host into the DoubleRow pair layout. Host pre-scales
    Wq/Wk/Wv/W1 by 32 and W2 by 64; inverse scales fold into activation
    scale constants.
  - a leading dummy AllGather absorbs program-launch skew so the real q/v
    AllGathers handshake fast. All gather-input stores and gathered-output
    loads are issued from the GpSimd queue so the Sync queue (x/weight
    loads) never blocks behind a collective wait, and the collective
    triggers sequence naturally behind their input stores.
  - LayerNorms run as moment sums: the PSUM->SBUF evictions carry
    accum_out partials (sum x), a scalar-engine Square pass accumulates
    sum x^2, and the tiny per-partition algebra runs on vector. LN1 is
    interleaved into pass B's second half so its transposes slot into the
    PE stream while vector/scalar overlap the attention matmuls.
  - FFN1 streams W1 once (group-outer, s-half-inner). FFN2 runs 2 passes
    of 4 s-tiles (all 8 PSUM banks); W2 k-tiles 13..15 stay resident so
    each pass's chains START on them and the per-s-tile chain ends stagger
    on the last streamed tile, letting epilogues drain while the next
    pass's resident matmuls keep the PE warm.
"""

import sys

if "/opt/trn_rl_repo" not in sys.path:
    sys.path.insert(0, "/opt/trn_rl_repo")

import numpy as np
import ml_dtypes

import concourse.bacc as bacc
import concourse.mybir as mybir
import concourse.tile as tile
from concourse.alu_op_type import AluOpType
from concourse.masks import make_identity


AF = mybir.ActivationFunctionType
FP32 = mybir.dt.float32
BF16 = mybir.dt.bfloat16
FP8 = mybir.dt.float8e4
DR = mybir.MatmulPerfMode.DoubleRow

B, S, D, H = 2, 4096, 1024, 4096
N_CORES = 8
G = 4                 # cores per group (one group per batch element)
S_LOC = S // G        # sequence rows per core
P = 128               # SBUF partitions
NF = 512              # matmul moving free-dim (one fp32 PSUM bank)
DT = D // P           # 8 d-tiles
KP = DT // 2          # 4 k-subtile pairs over D
ST = S_LOC // P       # 8 s-tiles per core
TJ = S // P           # 32 global t-subtiles
HT = H // P           # 32 h-tiles
HG = 4                # FFN1 weight-streaming groups
HPG = HT // HG        # 8 h-tiles per group
W2R = 3               # resident W2 tail k-pair tiles (13, 14, 15)
W2S = HT // 2 - W2R   # streamed W2 k-pair tiles per pass (0..12)
EPS = 1e-5
SCL = 32.0            # host pre-scale on Wq/Wk/Wv/W1
SCL2 = 64.0           # host pre-scale on W2
SM_SCALE = 1.0 / float(np.sqrt(np.float32(D)))
EXP_SCALE = SM_SCALE / (SCL * SCL)

GROUPS = [[0, 1, 2, 3], [4, 5, 6, 7]]


def build_graph(nc, tc, ext, trivial_gb):
    stream = ext["stream"]
    persist = ext["persist"]
    stage = ext["stage"]
    const = ext["const"]
    dram = ext["dram"]
    tcx = ext["tc"]

    # ---- constants ----
    ident_bf = const.tile([P, P], BF16, tag="ident_bf", name="ident_bf")
    make_identity(nc, ident_bf[:])
    ident_f = const.tile([P, P], FP32, tag="ident_f", name="ident_f")
    make_identity(nc, ident_f[:])
    ones_dr = const.tile([P, 2, 16], FP8, tag="ones_dr", name="ones_dr")
    nc.vector.memset(ones_dr[:, :, :], 1.0)
    ones_f32 = const.tile([1, P], FP32, tag="ones_f32", name="ones_f32")
    nc.vector.memset(ones_f32[:], 1.0)
    eps_t = const.tile([P, 1], FP32, tag="eps", name="eps")
    nc.vector.memset(eps_t[:], EPS)

    # biases arrive host-packed: [P, 8] bq | [P, 8] bk | [P, 32] b1.
    # (Their DMAs are issued after the first x loads so the cold-start x
    # chunk leads the Sync queue.)
    pvecs = const.tile([P, 48], FP32, tag="pvecs", name="pvecs")
    bq_sb = [pvecs[:, m:m + 1] for m in range(DT)]
    bk_sb = [pvecs[:, DT + m:DT + m + 1] for m in range(DT)]
    b1_sb = [pvecs[:, 2 * DT + m:2 * DT + m + 1] for m in range(HT)]

    # free-dim [1, D] rows at 32-aligned partitions (matmul-legal bases);
    # row 96 is scratch for the softmax recip row (never a matmul operand)
    smalls = const.tile([P, D], FP32, tag="smalls", name="smalls")
    SROW = {"bv": 0, "gamma": 32, "beta": 64}
    smalls2 = const.tile([1, D], FP32, tag="smalls2", name="smalls2")

    def load_small_consts():
        nc.sync.dma_start(out=pvecs[:, 0:DT], in_=ext["bqp_ext"][:, :])
        nc.sync.dma_start(out=pvecs[:, DT:2 * DT], in_=ext["bkp_ext"][:, :])
        nc.sync.dma_start(out=pvecs[:, 2 * DT:2 * DT + HT], in_=ext["b1p_ext"][:, :])
        for nm, r in SROW.items():
            nc.sync.dma_start(out=smalls[r:r + 1, :], in_=ext[nm + "_ext"][0:1, :])
        nc.sync.dma_start(out=smalls2[0:1, :], in_=ext["beta_b2_ext"][0:1, :])

    res = [persist.tile([P, D], FP32, tag=f"res{m}", name=f"res{m}") for m in range(ST)]
    xT_f8 = persist.tile([P, DT, S_LOC], FP8, tag="xT", name="xT")
    qT_f8 = persist.tile([P, DT, S_LOC], FP8, tag="qT", name="qT")
    kT_f8 = persist.tile([P, DT, S_LOC], FP8, tag="kT", name="kT")
    v_half = [persist.tile([P, TJ, NF], FP8, tag=f"vf{h}", name=f"vf{h}")
              for h in range(2)]
    P_f8 = persist.tile([P, TJ, S_LOC], FP8, tag="pf", name="pf")
    # resident W2 tail k-pair tiles (loaded during phase A, used by both
    # FFN2 passes), and FFN1's last weight group (its streamed slot would
    # WAR-stall against group 0's matmuls right at the FFN1->FFN2 seam)
    w2r = [persist.tile([P, 2, D], FP8, tag=f"w2r{i}", name=f"w2r{i}")
           for i in range(W2R)]
    w1g3r = [persist.tile([P, 2, D], FP8, tag=f"w1g3r{i}", name=f"w1g3r{i}")
             for i in range(KP)]

    # gather buffers are partition-major ([p, tile, col] flattened) so the
    # post-gather loads are single DMAs with 4KB-contiguous runs. q gathers
    # are split into quarter-meshes (per d-half, per 256-col t-quarter) so
    # the first gathered data lands one short mesh after the skew gate and
    # pass A can start ~a full mesh earlier.
    NQ = NF // 2          # 256 t-cols per q quarter-mesh
    ag_q_in = [[dram.tile([P, DT * NQ], FP8, name=f"agqi{h}_{qd}")
                for qd in range(2)] for h in range(2)]
    ag_q_out = [[dram.tile([G * P, DT * NQ], FP8, name=f"agqo{h}_{qd}")
                 for qd in range(2)] for h in range(2)]
    ag_v_in = [dram.tile([P, ST * NF], FP8, name=f"agvi{h}") for h in range(2)]
    ag_v_out = [dram.tile([G * P, ST * NF], FP8, name=f"agvo{h}") for h in range(2)]

    bcast = {}
    recipT = const.tile([P, ST], FP32, tag="recipT", name="recipT")

    def load_w8(ext_t, base_row, eng=None):
        eng = eng or nc.sync
        tiles = []
        for kp in range(KP):
            wt = stream.tile([P, 2, D], FP8, tag=f"w{kp}", name=f"w{kp}")
            r0 = base_row + kp * P
            eng.dma_start(out=wt[:, :, :], in_=ext_t[r0:r0 + P, :])
            tiles.append(wt)
        return tiles

    # LayerNorm via moments: sums arrive via accum_out on the evictions
    # (s01), sum-of-squares via a scalar-engine Square pass. Returns
    # (negmu*sd, sd) for the fused apply  (x + nm/sd)*sd = (x-mu)*sd.
    def ln_moments(st, s01):
        lt = stage.tile([P, 8], FP32, tag="lnt", name="lnt", bufs=3)
        sq = stage.tile([P, D], FP32, tag="sqt", name="sqt", bufs=1)
        sumsq = lt[:, 0:1]
        nc.scalar.activation(sq[:], res[st][:], AF.Square, accum_out=sumsq)
        ssum = lt[:, 1:2]
        nc.vector.tensor_add(ssum, s01[:, 0:1], s01[:, 1:2])
        negmu = lt[:, 2:3]
        nc.vector.tensor_scalar_mul(negmu, ssum, -1.0 / D)
        var = lt[:, 3:4]
        mu2 = lt[:, 4:5]
        nc.vector.tensor_mul(mu2, negmu, negmu)
        nc.vector.scalar_tensor_tensor(
            out=var, in0=sumsq, scalar=1.0 / D, in1=mu2,
            op0=AluOpType.mult, op1=AluOpType.subtract,
        )
        sd = lt[:, 5:6]
        nc.scalar.activation(sd, var, AF.Sqrt, bias=eps_t[:])
        nc.vector.reciprocal(sd, sd)
        nm = lt[:, 6:7]
        nc.vector.tensor_mul(nm, negmu, sd)
        return nm, sd

    def transpose_to(mmp, src_bf, dst_f8, s0):
        tp = mmp.tile([P, DT * P], BF16, tag="trp", name="trp", bufs=2)
        for dj in range(DT):
            nc.tensor.transpose(
                tp[:, dj * P:(dj + 1) * P], src_bf[:, dj * P:(dj + 1) * P],
                ident_bf[:],
            )
        nc.vector.tensor_copy(
            out=dst_f8[:, :, s0:s0 + P],
            in_=tp[:].rearrange("p (d s) -> p d s", d=DT),
        )

    # ================= phase A: QKV, attention, LN1, FFN1 =================
    with tcx.tile_pool(name="psA", bufs=1, space="PSUM") as mmp:
        # ---- x -> xT fp8: first s-half, then q-half0 can go ----
        # loads alternate between the Sync and Scalar queues so the 4MB of
        # x doesn't trickle through a single DMA ring at cold start
        x_eng = [nc.sync, nc.scalar, nc.sync, nc.scalar]

        def load_x_half(h):
            for si in range(h * 4, h * 4 + 4):
                xn = stage.tile([P, D], FP32, tag="stgf", name="stgf", bufs=3)
                xb = stage.tile([P, D], BF16, tag="stgb", name="stgb")
                eng = x_eng[si % 4]
                if si == 0:
                    # split the first tile so the cold-start PE work begins
                    # after ~256KB instead of ~512KB of DMA
                    for c in range(2):
                        eng.dma_start(
                            out=xn[:, c * NF:(c + 1) * NF],
                            in_=ext["x_ext"][si * P:(si + 1) * P, c * NF:(c + 1) * NF],
                        )
                        nc.vector.tensor_copy(
                            out=xb[:, c * NF:(c + 1) * NF],
                            in_=xn[:, c * NF:(c + 1) * NF],
                        )
                else:
                    eng.dma_start(out=xn[:], in_=ext["x_ext"][si * P:(si + 1) * P, :])
                    nc.vector.tensor_copy(out=xb[:], in_=xn[:])
                transpose_to(mmp, xb, xT_f8, si * P)

        def q_half(h):
            n0 = h * NF
            for m in range(DT):
                pt = mmp.tile([P, NF], FP32, tag="mm", name="mm", bufs=4)
                for kp in range(KP):
                    nc.tensor.matmul(
                        pt[:], wq[kp][:, :, m * P:(m + 1) * P],
                        xT_f8[:, 2 * kp:2 * kp + 2, n0:n0 + NF],
                        start=(kp == 0), stop=(kp == KP - 1), perf_mode=DR,
                    )
                nc.scalar.activation(qT_f8[:, m, n0:n0 + NF], pt[:], AF.Identity,
                                     bias=bq_sb[m])
                for qd in range(2):
                    nc.gpsimd.dma_start(
                        out=ag_q_in[h][qd][:, m * NQ:(m + 1) * NQ],
                        in_=qT_f8[:, m, n0 + qd * NQ:n0 + (qd + 1) * NQ],
                    )
            for qd in range(2):
                nc.gpsimd.collective_compute(
                    "AllGather", AluOpType.bypass, replica_groups=GROUPS,
                    ins=[ag_q_in[h][qd][:].opt()],
                    outs=[ag_q_out[h][qd][:].opt()],
                )

        wq = load_w8(ext["wq8_ext"], 0, eng=nc.scalar)
        load_x_half(0)
        load_small_consts()
        q_half(0)
        load_x_half(1)
        q_half(1)

        # ---- v = x @ (32 Wv) + 32 bv (natural, fp8); AllGather ----
        wv = load_w8(ext["wv8_ext"], 0)
        bv_b = const.tile([P, D], FP32, tag="bc_bv", name="bc_bv")
        for n0 in range(0, D, NF):
            pt = mmp.tile([P, NF], FP32, tag="mm", name="mm", bufs=4)
            nc.tensor.matmul(pt[:], ones_f32[0:1, :], smalls[0:1, n0:n0 + NF])
            nc.scalar.copy(out=bv_b[:, n0:n0 + NF], in_=pt[:])
        for mt in range(ST):
            v8 = stage.tile([P, D], FP8, tag="v8", name="v8")
            for n0 in range(0, D, NF):
                pt = mmp.tile([P, NF], FP32, tag="mm", name="mm", bufs=4)
                for kp in range(KP):
                    nc.tensor.matmul(
                        pt[:], xT_f8[:, 2 * kp:2 * kp + 2, mt * P:(mt + 1) * P],
                        wv[kp][:, :, n0:n0 + NF],
                        start=(kp == 0), stop=(kp == KP - 1), perf_mode=DR,
                    )
                nc.vector.tensor_add(
                    v8[:, n0:n0 + NF], pt[:], bv_b[:, n0:n0 + NF]
                )
            for hh in range(2):
                nc.gpsimd.dma_start(
                    out=ag_v_in[hh][:, mt * NF:(mt + 1) * NF],
                    in_=v8[:, hh * NF:(hh + 1) * NF],
                )
        # v gathered in two d-halves so pass B's first half can start while
        # the second half is still on the wire (the CC stream is serial)
        for hh in range(2):
            nc.gpsimd.collective_compute(
                "AllGather", AluOpType.bypass, replica_groups=GROUPS,
                ins=[ag_v_in[hh][:].opt()], outs=[ag_v_out[hh][:].opt()],
            )

        # ---- kT = (32 Wk).T @ x + 32 bk (fp8, local) ----
        wk = load_w8(ext["wk8_ext"], 0)
        # resident W2 tail tiles + FFN1's last weight group: load now (DMA
        # quiet spot; consumed in FFN1/FFN2)
        for i in range(W2R):
            kp2 = W2S + i
            nc.sync.dma_start(
                out=w2r[i][:, :, :], in_=ext["w28_ext"][kp2 * P:(kp2 + 1) * P, :]
            )
        for kp in range(KP):
            r0 = ((HG - 1) * KP + kp) * P
            nc.sync.dma_start(
                out=w1g3r[kp][:, :, :], in_=ext["w18_ext"][r0:r0 + P, :]
            )
        for m in range(DT):
            for n0 in range(0, S_LOC, NF):
                pt = mmp.tile([P, NF], FP32, tag="mm", name="mm", bufs=4)
                for kp in range(KP):
                    nc.tensor.matmul(
                        pt[:], wk[kp][:, :, m * P:(m + 1) * P],
                        xT_f8[:, 2 * kp:2 * kp + 2, n0:n0 + NF],
                        start=(kp == 0), stop=(kp == KP - 1), perf_mode=DR,
                    )
                nc.scalar.activation(kT_f8[:, m, n0:n0 + NF], pt[:], AF.Identity,
                                     bias=bk_sb[m])

        # [P, D] broadcasts, off the critical path (fills AG wait)
        bc_rows = [("gamma", smalls[32:33, :]),
                   ("beta", smalls[64:65, :]),
                   ("beta_b2", smalls2[0:1, :])]
        if trivial_gb:
            bc_rows = [bc_rows[2]]  # only beta+b2 needed
        for nm, srow in bc_rows:
            bt = const.tile([P, D], FP32, tag=f"bc_{nm}", name=f"bc_{nm}")
            for n0 in range(0, D, NF):
                pt = mmp.tile([P, NF], FP32, tag="mm", name="mm", bufs=4)
                nc.tensor.matmul(pt[:], ones_f32[0:1, :], srow[:, n0:n0 + NF])
                nc.scalar.copy(out=bt[:, n0:n0 + NF], in_=pt[:])
            bcast[nm] = bt

        # ---- pass A: P[t, s] = exp(k·q/sqrt(D)); DR rowsums 1 chunk back ----
        rs_ps = [mmp.tile([1, NF], FP32, tag=f"rs{h}", name=f"rs{h}", bufs=1)
                 for h in range(2)]
        chunks = [(ht, qd, r) for ht in range(2) for qd in range(2)
                  for r in range(G)]

        def emit_rowsum(ci):
            ht, qd, r = chunks[ci]
            jp0 = (r * ST + ht * 4 + qd * 2) // 2
            for h in range(2):
                n0 = h * NF
                nc.tensor.matmul(
                    rs_ps[h][:], ones_dr[:, :, 0:1],
                    P_f8[:, 2 * jp0:2 * jp0 + 2, n0:n0 + NF],
                    start=(ci == 0), stop=(ci == len(chunks) - 1),
                    perf_mode=DR,
                )

        qtiles = {}

        def issue_qch(ci):
            ht, qd, r = chunks[ci]
            qch = stream.tile([P, DT, NQ], FP8, tag="q", name="q", bufs=4)
            nc.sync.dma_start(
                out=qch[:, :, :], in_=ag_q_out[ht][qd][r * P:(r + 1) * P, :]
            )
            qtiles[ci] = qch

        issue_qch(0)
        issue_qch(1)
        issue_qch(2)
        for ci, (ht, qd, r) in enumerate(chunks):
            if ci + 3 < len(chunks):
                issue_qch(ci + 3)
            qch = qtiles.pop(ci)
            for tti in range(2):
                j = r * ST + ht * 4 + qd * 2 + tti
                for n0 in range(0, S_LOC, NF):
                    ps = mmp.tile([P, NF], FP32, tag="mm", name="mm", bufs=4)
                    for kp in range(KP):
                        nc.tensor.matmul(
                            ps[:], qch[:, 2 * kp:2 * kp + 2, tti * P:(tti + 1) * P],
                            kT_f8[:, 2 * kp:2 * kp + 2, n0:n0 + NF],
                            start=(kp == 0), stop=(kp == KP - 1), perf_mode=DR,
                        )
                    nc.scalar.activation(
                        P_f8[:, j, n0:n0 + NF], ps[:], AF.Exp, scale=EXP_SCALE
                    )
            if ci > 0:
                emit_rowsum(ci - 1)
        emit_rowsum(len(chunks) - 1)

        # raw rowsums -> smalls row 96 (scalar copies: a [1,512] vector op
        # is single-partition and takes ~3.3us); the reciprocal runs after
        # the transpose in [P, 8] form where it's ~100x faster. The tiny
        # transpose is emitted inside pass B (after st0's matmuls) so the
        # PE queue doesn't stall on it before the attention matmuls.
        rs_row = smalls[96:97, :]
        for h in range(2):
            nc.scalar.copy(out=rs_row[0:1, h * NF:(h + 1) * NF], in_=rs_ps[h][:])
        rs8 = const.tile([ST, P], FP32, tag="rs8", name="rs8")
        nc.scalar.dma_start(out=rs8[:, :], in_=rs_row[0:1, :])

        # ---- pass B: attn natural [s, d] + residual -> res (fp32) ----
        # d-half outer: half 0 computes while v's half-1 gather is in flight.
        # gathered-v loads ride the GpSimd queue (idle after the triggers) so
        # the Sync queue's qch stream can't delay them.
        for hh in range(2):
            nc.gpsimd.dma_start(
                out=v_half[hh][:, :, :].rearrange("p (r m) c -> p r (m c)", r=G),
                in_=ag_v_out[hh][:, :].rearrange("(r p) c -> p r c", p=P),
            )

        # LN1 interleaved into the h=1 evictions: stats/apply/cast overlap
        # the next s-tile's attention matmuls; transposes slot into the PE
        # stream.
        x1T_f8 = persist.tile([P, DT, S_LOC], FP8, tag="xT", name="xT")
        h_sh = [persist.tile([P, TJ, NF], FP8, tag=f"vf{h}", name=f"vf{h}")
                for h in range(2)]

        s01 = {}
        for h in range(2):
            n0 = h * NF
            for st in range(ST):
                xre = stage.tile([P, NF], FP32, tag="xre", name="xre")
                nc.scalar.dma_start(
                    out=xre[:], in_=ext["x_ext"][st * P:(st + 1) * P, n0:n0 + NF]
                )
                ps = mmp.tile([P, NF], FP32, tag="mm", name="mm", bufs=4)
                for jp in range(TJ // 2):
                    nc.tensor.matmul(
                        ps[:], P_f8[:, 2 * jp:2 * jp + 2, st * P:(st + 1) * P],
                        v_half[h][:, 2 * jp:2 * jp + 2, :],
                        start=(jp == 0), stop=(jp == TJ // 2 - 1), perf_mode=DR,
                    )
                if h == 0 and st == 0:
                    rt_ps = mmp.tile([P, NF], FP32, tag="mm", name="mm", bufs=4)
                    nc.tensor.transpose(rt_ps[:, 0:ST], rs8[:, :],
                                        ident_f[0:ST, 0:ST])
                    nc.scalar.activation(recipT[:], rt_ps[:, 0:ST], AF.Identity,
                                         scale=SCL)
                    nc.vector.reciprocal(recipT[:], recipT[:])
                if h == 0:
                    sx = stage.tile([P, 2], FP32, tag="s01", name="s01", bufs=8)
                    s01[st] = sx
                    acc = sx[:, 0:1]
                else:
                    acc = s01[st][:, 1:2]
                nc.vector.scalar_tensor_tensor(
                    out=res[st][:, n0:n0 + NF], in0=ps[:], scalar=recipT[:, st:st + 1],
                    in1=xre[:], op0=AluOpType.mult, op1=AluOpType.add,
                    accum_out=acc,
                )
                if h == 1:
                    # LN1: res[st] <- z (normalized); then cast+transpose
                    nm, sd = ln_moments(st, s01[st])
                    nc.vector.tensor_scalar(
                        res[st][:], res[st][:], nm, sd,
                        op0=AluOpType.add, op1=AluOpType.mult,
                    )
                    xb = stage.tile([P, D], BF16, tag="stgb", name="stgb")
                    nc.scalar.copy(out=xb[:], in_=res[st][:])
                    transpose_to(mmp, xb, x1T_f8, st * P)

        # ---- FFN1: stream W1 once (group-outer, s-half-inner) ----
        for g in range(HG):
            w1g = (w1g3r if g == HG - 1
                   else load_w8(ext["w18_ext"], g * KP * P))
            for sh in range(2):
                n0 = sh * NF
                for mh_i in range(HPG):
                    mh = g * HPG + mh_i
                    pt = mmp.tile([P, NF], FP32, tag="mm", name="mm", bufs=4)
                    for kp in range(KP):
                        nc.tensor.matmul(
                            pt[:], w1g[kp][:, :, mh_i * P:(mh_i + 1) * P],
                            x1T_f8[:, 2 * kp:2 * kp + 2, n0:n0 + NF],
                            start=(kp == 0), stop=(kp == KP - 1), perf_mode=DR,
                        )
                    nc.scalar.activation(
                        h_sh[sh][:, mh, :], pt[:], AF.Gelu,
                        bias=b1_sb[mh], scale=1.0 / SCL,
                    )

    # ================= phase B: FFN2 (fp8 DR) + LN2 + out =================
    # 2 passes of 4 s-tiles (8 PSUM banks). Chains START on the resident
    # W2 tail tiles, then consume streamed k 0..12; per-s-tile chain ends
    # stagger on k=12 so the epilogues pipeline while pass 2's resident
    # matmuls keep the PE warm.
    with tcx.tile_pool(name="psB", bufs=1, space="PSUM") as f2p:
        for sp, sts in enumerate([(0, 1, 2, 3), (4, 5, 6, 7)]):
            f2 = {(st, h): f2p.tile([P, NF], FP32, tag=f"f{st % 4}_{h}",
                                    name=f"f{st % 4}_{h}")
                  for st in sts for h in range(2)}

            def f2mm(kp2, st, h, w2t, start=False, stop=False):
                nc.tensor.matmul(
                    f2[(st, h)][:],
                    h_sh[st // 4][:, 2 * kp2:2 * kp2 + 2,
                                  (st % 4) * P:(st % 4 + 1) * P],
                    w2t[:, :, h * NF:(h + 1) * NF],
                    start=start, stop=stop, perf_mode=DR,
                )

            # resident tail first: starts the accumulation groups
            for st in sts:
                for h in range(2):
                    for i in range(W2R):
                        f2mm(W2S + i, st, h, w2r[i], start=(i == 0))

            # streamed k-pairs 0..W2S-1; stagger chain ends on the last one
            for kp2 in range(W2S):
                wt = stream.tile([P, 2, D], FP8, tag=f"w{kp2 % KP}",
                                 name=f"w{kp2 % KP}")
                nc.sync.dma_start(
                    out=wt[:, :, :],
                    in_=ext["w28_ext"][kp2 * P:(kp2 + 1) * P, :],
                )
                if kp2 == 0 and trivial_gb:
                    # pre-add beta+b2 into res here (vector is idle during
                    # the matmul stream) so the per-tile tail epilogue shrinks
                    for st in sts:
                        nc.vector.tensor_add(res[st][:], res[st][:],
                                             bcast["beta_b2"][:])
                if kp2 < W2S - 1:
                    for st in sts:
                        for h in range(2):
                            f2mm(kp2, st, h, wt)
                else:
                    for st in sts:
                        for h in range(2):
                            f2mm(kp2, st, h, wt, stop=True)
                        # epilogue streams per s-tile as its chains end
                        s2 = stage.tile([P, 2], FP32, tag="s01", name="s01",
                                        bufs=8)
                        if trivial_gb:
                            # pre-LN2 = z + (beta+b2) + f2/SCL2 (beta_b2
                            # pre-added above)
                            for h in range(2):
                                n0 = h * NF
                                nc.vector.scalar_tensor_tensor(
                                    out=res[st][:, n0:n0 + NF], in0=f2[(st, h)][:],
                                    scalar=1.0 / SCL2, in1=res[st][:, n0:n0 + NF],
                                    op0=AluOpType.mult, op1=AluOpType.add,
                                    accum_out=s2[:, h:h + 1],
                                )
                        else:
                            t2 = stage.tile([P, D], FP32, tag="stgf2", name="stgf2")
                            nc.vector.tensor_mul(t2[:], res[st][:], bcast["gamma"][:])
                            for h in range(2):
                                n0 = h * NF
                                nc.vector.scalar_tensor_tensor(
                                    out=t2[:, n0:n0 + NF], in0=f2[(st, h)][:],
                                    scalar=1.0 / SCL2, in1=t2[:, n0:n0 + NF],
                                    op0=AluOpType.mult, op1=AluOpType.add,
                                )
                            nc.vector.tensor_add(res[st][:], t2[:],
                                                 bcast["beta_b2"][:])
                            nc.vector.reduce_sum(s2[:, 0:1], res[st][:, 0:NF])
                            nc.vector.reduce_sum(s2[:, 1:2], res[st][:, NF:D])
                        # LN2 + store
                        nm, sd = ln_moments(st, s2)
                        ot = stage.tile([P, D], FP32, tag="stgf", name="stgf",
                                        bufs=3)
                        nc.vector.tensor_scalar(
                            ot[:], res[st][:], nm, sd,
                            op0=AluOpType.add, op1=AluOpType.mult,
                        )
                        if not trivial_gb:
                            nc.vector.tensor_mul(ot[:], ot[:], bcast["gamma"][:])
                            nc.vector.tensor_add(ot[:], ot[:], bcast["beta"][:])
                        nc.gpsimd.dma_start(
                            out=ext["out_ext"][st * P:(st + 1) * P, :], in_=ot[:]
                        )


def build_nc(trivial_gb):
    nc = bacc.Bacc(target_bir_lowering=False, num_devices=N_CORES)

    ext = {
        "x_ext": nc.declare_dram_parameter("x", [S_LOC, D], FP32, isOutput=False),
        "wq8_ext": nc.declare_dram_parameter("wq8", [KP * P, 2 * D], FP8, isOutput=False),
        "wk8_ext": nc.declare_dram_parameter("wk8", [KP * P, 2 * D], FP8, isOutput=False),
        "wv8_ext": nc.declare_dram_parameter("wv8", [KP * P, 2 * D], FP8, isOutput=False),
        "w18_ext": nc.declare_dram_parameter("w18", [HG * KP * P, 2 * D], FP8, isOutput=False),
        "w28_ext": nc.declare_dram_parameter("w28", [(HT // 2) * P, 2 * D], FP8, isOutput=False),
        "bqp_ext": nc.declare_dram_parameter("bqp", [P, DT], FP32, isOutput=False),
        "bkp_ext": nc.declare_dram_parameter("bkp", [P, DT], FP32, isOutput=False),
        "b1p_ext": nc.declare_dram_parameter("b1p", [P, HT], FP32, isOutput=False),
        "bv_ext": nc.declare_dram_parameter("bv", [1, D], FP32, isOutput=False),
        "beta_b2_ext": nc.declare_dram_parameter("beta_b2", [1, D], FP32, isOutput=False),
        "gamma_ext": nc.declare_dram_parameter("gamma", [1, D], FP32, isOutput=False),
        "beta_ext": nc.declare_dram_parameter("beta", [1, D], FP32, isOutput=False),
        "out_ext": nc.declare_dram_parameter("out", [S_LOC, D], FP32, isOutput=True),
    }

    with tile.TileContext(nc) as tc:
        with (
            tc.tile_pool(name="dram", bufs=1, space="DRAM") as dram,
            tc.tile_pool(name="const", bufs=1) as const,
            tc.tile_pool(name="persist", bufs=1) as persist,
            tc.tile_pool(name="stage", bufs=2) as stage,
            tc.tile_pool(name="stream", bufs=2) as stream,
        ):
            ext.update(tc=tc, dram=dram, const=const, persist=persist,
                       stage=stage, stream=stream)
            build_graph(nc, tc, ext, trivial_gb)
    nc.compile()
    return nc


_NC_CACHE = {}


def _get_nc(trivial_gb):
    if trivial_gb not in _NC_CACHE:
        _NC_CACHE[trivial_gb] = build_nc(trivial_gb)
    return _NC_CACHE[trivial_gb]


F8NP = ml_dtypes.float8_e4m3


def _pair_rows(w):
    # [K, N] -> pair layout: rows kp*128+p, cols i*N+c = w[(2kp+i)*128+p, c]
    k, n = w.shape
    kp = k // (2 * P)
    w4 = w.reshape(kp, 2, P, n).transpose(0, 2, 1, 3).reshape(kp * P, 2 * n)
    return np.ascontiguousarray(w4)


def _col_pack(v, n):
    # [n*128] -> [128, n] with out[p, m] = v[m*128 + p]
    return np.ascontiguousarray(v.reshape(n, P).T)


def _make_in_maps(inputs):
    x = np.asarray(inputs["input_embedding"], dtype=np.float32)
    assert x.shape == (B, S, D), x.shape

    gamma = np.asarray(inputs["gamma"], np.float32).reshape(D)
    beta = np.asarray(inputs["beta"], np.float32).reshape(D)
    trivial_gb = bool(np.all(gamma == 1.0) and np.all(beta == 0.0))
    W1 = np.asarray(inputs["W1"], np.float32)
    b1 = np.asarray(inputs["b1"], np.float32).reshape(H)
    # fold LN1's gamma/beta into W1/b1 (FFN1 consumes the normalized z)
    W1f = gamma[:, None] * W1
    b1f = b1 + beta @ W1
    # W1 group-major pair layout: rows (g*KP+kp)*128+p, cols i*D+c
    w1g = (SCL * W1f).reshape(KP, 2, P, HG, D).transpose(3, 0, 2, 1, 4)
    w18 = np.ascontiguousarray(w1g.reshape(HG * KP * P, 2 * D)).astype(F8NP)

    shared = {
        "wq8": _pair_rows(SCL * np.asarray(inputs["Wq"], np.float32)).astype(F8NP),
        "wk8": _pair_rows(SCL * np.asarray(inputs["Wk"], np.float32)).astype(F8NP),
        "wv8": _pair_rows(SCL * np.asarray(inputs["Wv"], np.float32)).astype(F8NP),
        "w18": w18,
        "w28": _pair_rows(SCL2 * np.asarray(inputs["W2"], np.float32)).astype(F8NP),
        "bqp": _col_pack(SCL * np.asarray(inputs["bq"], np.float32).reshape(D), DT),
        "bkp": _col_pack(SCL * np.asarray(inputs["bk"], np.float32).reshape(D), DT),
        "b1p": _col_pack(b1f, HT),
        "bv": SCL * np.asarray(inputs["bv"], np.float32).reshape(1, D),
        "beta_b2": (beta + np.asarray(inputs["b2"], np.float32).reshape(D)).reshape(1, D),
        "gamma": gamma.reshape(1, D),
        "beta": beta.reshape(1, D),
    }

    in_maps = []
    for c in range(N_CORES):
        b = c // G
        r = c % G
        m = dict(shared)
        m["x"] = np.ascontiguousarray(x[b, r * S_LOC:(r + 1) * S_LOC, :])
        in_maps.append(m)
    return in_maps, trivial_gb


def kernel(**inputs: np.ndarray) -> np.ndarray:
    from concourse.bass_utils import run_bass_kernel_spmd

    in_maps, trivial_gb = _make_in_maps(inputs)
    nc = _get_nc(trivial_gb)
    res = run_bass_kernel_spmd(nc, in_maps, core_ids=list(range(N_CORES)))

    out = np.empty((B, S, D), dtype=np.float32)
    for c in range(N_CORES):
        b = c // G
        r = c % G
        out[b, r * S_LOC:(r + 1) * S_LOC, :] = res.results[c]["out"]
    return out


# revision 37
# speedup vs baseline: 1.1305x; 1.1305x over previous
"""Distributed Trainium2 kernel for a transformer attention block (B=2, S=4096,
D=1024, H=4096, fp32 I/O).

Reference computation (note the Q<-k, K<-q, V<-v argument quirk):
    k = x @ Wk + bk ; q = x @ Wq + bq ; v = x @ Wv + bv
    scores[s,t] = k[s]·q[t] / sqrt(D); attn = softmax_t(scores) @ v
    x1 = LN(x + attn); h = gelu(x1 @ W1 + b1); out = LN(x1 + h @ W2 + b2)

Sharding: 8 cores -> 2 groups of 4 (one group per batch element); each core
owns 1024 sequence rows. Design notes:
  - all five GEMMs run fp8 (e4m3) DoubleRow matmuls. Weights are pre-cast/
    pre-tiled on the host into the DoubleRow pair layout. Host pre-scales
    Wq/Wk/Wv/W1 by 32 and W2 by 64; inverse scales fold into activation
    scale constants.
  - a leading dummy AllGather absorbs program-launch skew so the real q/v
    AllGathers handshake fast. All gather-input stores and gathered-output
    loads are issued from the GpSimd queue so the Sync queue (x/weight
    loads) never blocks behind a collective wait, and the collective
    triggers sequence naturally behind their input stores.
  - LayerNorms run as moment sums: the PSUM->SBUF evictions carry
    accum_out partials (sum x), a scalar-engine Square pass accumulates
    sum x^2, and the tiny per-partition algebra runs on vector. LN1 is
    interleaved into pass B's second half so its transposes slot into the
    PE stream while vector/scalar overlap the attention matmuls.
  - FFN1 streams W1 once (group-outer, s-half-inner). FFN2 runs 2 passes
    of 4 s-tiles (all 8 PSUM banks); W2 k-tiles 13..15 stay resident so
    each pass's chains START on them and the per-s-tile chain ends stagger
    on the last streamed tile, letting epilogues drain while the next
    pass's resident matmuls keep the PE warm.
"""

import sys

if "/opt/trn_rl_repo" not in sys.path:
    sys.path.insert(0, "/opt/trn_rl_repo")

import numpy as np
import ml_dtypes

import concourse.bacc as bacc
import concourse.mybir as mybir
import concourse.tile as tile
from concourse.alu_op_type import AluOpType
from concourse.masks import make_identity


AF = mybir.ActivationFunctionType
FP32 = mybir.dt.float32
BF16 = mybir.dt.bfloat16
FP8 = mybir.dt.float8e4
DR = mybir.MatmulPerfMode.DoubleRow

B, S, D, H = 2, 4096, 1024, 4096
N_CORES = 8
G = 4                 # cores per group (one group per batch element)
S_LOC = S // G        # sequence rows per core
P = 128               # SBUF partitions
NF = 512              # matmul moving free-dim (one fp32 PSUM bank)
DT = D // P           # 8 d-tiles
KP = DT // 2          # 4 k-subtile pairs over D
ST = S_LOC // P       # 8 s-tiles per core
TJ = S // P           # 32 global t-subtiles
HT = H // P           # 32 h-tiles
HG = 4                # FFN1 weight-streaming groups
HPG = HT // HG        # 8 h-tiles per group
W2R = 3               # resident W2 tail k-pair tiles (13, 14, 15)
W2S = HT // 2 - W2R   # streamed W2 k-pair tiles per pass (0..12)
EPS = 1e-5
SCL = 32.0            # host pre-scale on Wq/Wk/Wv/W1
SCL2 = 64.0           # host pre-scale on W2
SM_SCALE = 1.0 / float(np.sqrt(np.float32(D)))
EXP_SCALE = SM_SCALE / (SCL * SCL)

GROUPS = [[0, 1, 2, 3], [4, 5, 6, 7]]


def build_graph(nc, tc, ext, trivial_gb):
    stream = ext["stream"]
    persist = ext["persist"]
    stage = ext["stage"]
    const = ext["const"]
    dram = ext["dram"]
    tcx = ext["tc"]

    # ---- constants ----
    ident_bf = const.tile([P, P], BF16, tag="ident_bf", name="ident_bf")
    make_identity(nc, ident_bf[:])
    ident_f = const.tile([P, P], FP32, tag="ident_f", name="ident_f")
    make_identity(nc, ident_f[:])
    ones_dr = const.tile([P, 2, 16], FP8, tag="ones_dr", name="ones_dr")
    nc.vector.memset(ones_dr[:, :, :], 1.0)
    ones_f32 = const.tile([1, P], FP32, tag="ones_f32", name="ones_f32")
    nc.vector.memset(ones_f32[:], 1.0)
    eps_t = const.tile([P, 1], FP32, tag="eps", name="eps")
    nc.vector.memset(eps_t[:], EPS)

    # biases arrive host-packed: [P, 8] bq | [P, 8] bk | [P, 32] b1.
    # (Their DMAs are issued after the first x loads so the cold-start x
    # chunk leads the Sync queue.)
    pvecs = const.tile([P, 48], FP32, tag="pvecs", name="pvecs")
    bq_sb = [pvecs[:, m:m + 1] for m in range(DT)]
    bk_sb = [pvecs[:, DT + m:DT + m + 1] for m in range(DT)]
    b1_sb = [pvecs[:, 2 * DT + m:2 * DT + m + 1] for m in range(HT)]

    # free-dim [1, D] rows at 32-aligned partitions (matmul-legal bases);
    # row 96 is scratch for the softmax recip row (never a matmul operand)
    smalls = const.tile([P, D], FP32, tag="smalls", name="smalls")
    SROW = {"bv": 0, "gamma": 32, "beta": 64}
    smalls2 = const.tile([1, D], FP32, tag="smalls2", name="smalls2")

    def load_small_consts():
        nc.sync.dma_start(out=pvecs[:, 0:DT], in_=ext["bqp_ext"][:, :])
        nc.sync.dma_start(out=pvecs[:, DT:2 * DT], in_=ext["bkp_ext"][:, :])
        nc.sync.dma_start(out=pvecs[:, 2 * DT:2 * DT + HT], in_=ext["b1p_ext"][:, :])
        for nm, r in SROW.items():
            nc.sync.dma_start(out=smalls[r:r + 1, :], in_=ext[nm + "_ext"][0:1, :])
        nc.sync.dma_start(out=smalls2[0:1, :], in_=ext["beta_b2_ext"][0:1, :])

    res = [persist.tile([P, D], FP32, tag=f"res{m}", name=f"res{m}") for m in range(ST)]
    xT_f8 = persist.tile([P, DT, S_LOC], FP8, tag="xT", name="xT")
    qT_f8 = persist.tile([P, DT, S_LOC], FP8, tag="qT", name="qT")
    kT_f8 = persist.tile([P, DT, S_LOC], FP8, tag="kT", name="kT")
    v_half = [persist.tile([P, TJ, NF], FP8, tag=f"vf{h}", name=f"vf{h}")
              for h in range(2)]
    P_f8 = persist.tile([P, TJ, S_LOC], FP8, tag="pf", name="pf")
    # resident W2 tail k-pair tiles (loaded during phase A, used by both
    # FFN2 passes), and FFN1's last weight group (its streamed slot would
    # WAR-stall against group 0's matmuls right at the FFN1->FFN2 seam)
    w2r = [persist.tile([P, 2, D], FP8, tag=f"w2r{i}", name=f"w2r{i}")
           for i in range(W2R)]
    w1g3r = [persist.tile([P, 2, D], FP8, tag=f"w1g3r{i}", name=f"w1g3r{i}")
             for i in range(KP)]

    # gather buffers are partition-major ([p, tile, col] flattened) so the
    # post-gather loads are single DMAs with 4KB-contiguous runs
    ag_q_in = [dram.tile([P, DT * NF], FP8, name=f"agqi{h}") for h in range(2)]
    ag_q_out = [dram.tile([G * P, DT * NF], FP8, name=f"agqo{h}") for h in range(2)]
    ag_v_in = [dram.tile([P, ST * NF], FP8, name=f"agvi{h}") for h in range(2)]
    ag_v_out = [dram.tile([G * P, ST * NF], FP8, name=f"agvo{h}") for h in range(2)]

    bcast = {}
    recipT = const.tile([P, ST], FP32, tag="recipT", name="recipT")

    def load_w8(ext_t, base_row, eng=None):
        eng = eng or nc.sync
        tiles = []
        for kp in range(KP):
            wt = stream.tile([P, 2, D], FP8, tag=f"w{kp}", name=f"w{kp}")
            r0 = base_row + kp * P
            eng.dma_start(out=wt[:, :, :], in_=ext_t[r0:r0 + P, :])
            tiles.append(wt)
        return tiles

    # LayerNorm via moments: sums arrive via accum_out on the evictions
    # (s01), sum-of-squares via a scalar-engine Square pass. Returns
    # (negmu*sd, sd) for the fused apply  (x + nm/sd)*sd = (x-mu)*sd.
    def ln_moments(st, s01):
        lt = stage.tile([P, 8], FP32, tag="lnt", name="lnt", bufs=3)
        sq = stage.tile([P, D], FP32, tag="sqt", name="sqt", bufs=1)
        sumsq = lt[:, 0:1]
        nc.scalar.activation(sq[:], res[st][:], AF.Square, accum_out=sumsq)
        ssum = lt[:, 1:2]
        nc.vector.tensor_add(ssum, s01[:, 0:1], s01[:, 1:2])
        negmu = lt[:, 2:3]
        nc.vector.tensor_scalar_mul(negmu, ssum, -1.0 / D)
        var = lt[:, 3:4]
        mu2 = lt[:, 4:5]
        nc.vector.tensor_mul(mu2, negmu, negmu)
        nc.vector.scalar_tensor_tensor(
            out=var, in0=sumsq, scalar=1.0 / D, in1=mu2,
            op0=AluOpType.mult, op1=AluOpType.subtract,
        )
        sd = lt[:, 5:6]
        nc.scalar.activation(sd, var, AF.Sqrt, bias=eps_t[:])
        nc.vector.reciprocal(sd, sd)
        nm = lt[:, 6:7]
        nc.vector.tensor_mul(nm, negmu, sd)
        return nm, sd

    def transpose_to(mmp, src_bf, dst_f8, s0):
        tp = mmp.tile([P, DT * P], BF16, tag="trp", name="trp", bufs=2)
        for dj in range(DT):
            nc.tensor.transpose(
                tp[:, dj * P:(dj + 1) * P], src_bf[:, dj * P:(dj + 1) * P],
                ident_bf[:],
            )
        nc.vector.tensor_copy(
            out=dst_f8[:, :, s0:s0 + P],
            in_=tp[:].rearrange("p (d s) -> p d s", d=DT),
        )

    # ================= phase A: QKV, attention, LN1, FFN1 =================
    with tcx.tile_pool(name="psA", bufs=1, space="PSUM") as mmp:
        # ---- x -> xT fp8: first s-half, then q-half0 can go ----
        # loads alternate between the Sync and Scalar queues so the 4MB of
        # x doesn't trickle through a single DMA ring at cold start
        x_eng = [nc.sync, nc.scalar, nc.sync, nc.scalar]

        def load_x_half(h):
            for si in range(h * 4, h * 4 + 4):
                xn = stage.tile([P, D], FP32, tag="stgf", name="stgf", bufs=3)
                xb = stage.tile([P, D], BF16, tag="stgb", name="stgb")
                eng = x_eng[si % 4]
                if si == 0:
                    # split the first tile so the cold-start PE work begins
                    # after ~256KB instead of ~512KB of DMA
                    for c in range(2):
                        eng.dma_start(
                            out=xn[:, c * NF:(c + 1) * NF],
                            in_=ext["x_ext"][si * P:(si + 1) * P, c * NF:(c + 1) * NF],
                        )
                        nc.vector.tensor_copy(
                            out=xb[:, c * NF:(c + 1) * NF],
                            in_=xn[:, c * NF:(c + 1) * NF],
                        )
                else:
                    eng.dma_start(out=xn[:], in_=ext["x_ext"][si * P:(si + 1) * P, :])
                    nc.vector.tensor_copy(out=xb[:], in_=xn[:])
                transpose_to(mmp, xb, xT_f8, si * P)

        def q_half(h):
            n0 = h * NF
            for m in range(DT):
                pt = mmp.tile([P, NF], FP32, tag="mm", name="mm", bufs=4)
                for kp in range(KP):
                    nc.tensor.matmul(
                        pt[:], wq[kp][:, :, m * P:(m + 1) * P],
                        xT_f8[:, 2 * kp:2 * kp + 2, n0:n0 + NF],
                        start=(kp == 0), stop=(kp == KP - 1), perf_mode=DR,
                    )
                nc.scalar.activation(qT_f8[:, m, n0:n0 + NF], pt[:], AF.Identity,
                                     bias=bq_sb[m])
                nc.gpsimd.dma_start(
                    out=ag_q_in[h][:, m * NF:(m + 1) * NF],
                    in_=qT_f8[:, m, n0:n0 + NF],
                )
            nc.gpsimd.collective_compute(
                "AllGather", AluOpType.bypass, replica_groups=GROUPS,
                ins=[ag_q_in[h][:].opt()], outs=[ag_q_out[h][:].opt()],
            )

        wq = load_w8(ext["wq8_ext"], 0, eng=nc.scalar)
        load_x_half(0)
        load_small_consts()
        q_half(0)
        load_x_half(1)
        q_half(1)

        # ---- v = x @ (32 Wv) + 32 bv (natural, fp8); AllGather ----
        wv = load_w8(ext["wv8_ext"], 0)
        bv_b = const.tile([P, D], FP32, tag="bc_bv", name="bc_bv")
        for n0 in range(0, D, NF):
            pt = mmp.tile([P, NF], FP32, tag="mm", name="mm", bufs=4)
            nc.tensor.matmul(pt[:], ones_f32[0:1, :], smalls[0:1, n0:n0 + NF])
            nc.scalar.copy(out=bv_b[:, n0:n0 + NF], in_=pt[:])
        for mt in range(ST):
            v8 = stage.tile([P, D], FP8, tag="v8", name="v8")
            for n0 in range(0, D, NF):
                pt = mmp.tile([P, NF], FP32, tag="mm", name="mm", bufs=4)
                for kp in range(KP):
                    nc.tensor.matmul(
                        pt[:], xT_f8[:, 2 * kp:2 * kp + 2, mt * P:(mt + 1) * P],
                        wv[kp][:, :, n0:n0 + NF],
                        start=(kp == 0), stop=(kp == KP - 1), perf_mode=DR,
                    )
                nc.vector.tensor_add(
                    v8[:, n0:n0 + NF], pt[:], bv_b[:, n0:n0 + NF]
                )
            for hh in range(2):
                nc.gpsimd.dma_start(
                    out=ag_v_in[hh][:, mt * NF:(mt + 1) * NF],
                    in_=v8[:, hh * NF:(hh + 1) * NF],
                )
        # v gathered in two d-halves so pass B's first half can start while
        # the second half is still on the wire (the CC stream is serial)
        for hh in range(2):
            nc.gpsimd.collective_compute(
                "AllGather", AluOpType.bypass, replica_groups=GROUPS,
                ins=[ag_v_in[hh][:].opt()], outs=[ag_v_out[hh][:].opt()],
            )

        # ---- kT = (32 Wk).T @ x + 32 bk (fp8, local) ----
        wk = load_w8(ext["wk8_ext"], 0)
        # resident W2 tail tiles + FFN1's last weight group: load now (DMA
        # quiet spot; consumed in FFN1/FFN2)
        for i in range(W2R):
            kp2 = W2S + i
            nc.sync.dma_start(
                out=w2r[i][:, :, :], in_=ext["w28_ext"][kp2 * P:(kp2 + 1) * P, :]
            )
        for kp in range(KP):
            r0 = ((HG - 1) * KP + kp) * P
            nc.sync.dma_start(
                out=w1g3r[kp][:, :, :], in_=ext["w18_ext"][r0:r0 + P, :]
            )
        for m in range(DT):
            for n0 in range(0, S_LOC, NF):
                pt = mmp.tile([P, NF], FP32, tag="mm", name="mm", bufs=4)
                for kp in range(KP):
                    nc.tensor.matmul(
                        pt[:], wk[kp][:, :, m * P:(m + 1) * P],
                        xT_f8[:, 2 * kp:2 * kp + 2, n0:n0 + NF],
                        start=(kp == 0), stop=(kp == KP - 1), perf_mode=DR,
                    )
                nc.scalar.activation(kT_f8[:, m, n0:n0 + NF], pt[:], AF.Identity,
                                     bias=bk_sb[m])

        # [P, D] broadcasts, off the critical path (fills AG wait)
        bc_rows = [("gamma", smalls[32:33, :]),
                   ("beta", smalls[64:65, :]),
                   ("beta_b2", smalls2[0:1, :])]
        if trivial_gb:
            bc_rows = [bc_rows[2]]  # only beta+b2 needed
        for nm, srow in bc_rows:
            bt = const.tile([P, D], FP32, tag=f"bc_{nm}", name=f"bc_{nm}")
            for n0 in range(0, D, NF):
                pt = mmp.tile([P, NF], FP32, tag="mm", name="mm", bufs=4)
                nc.tensor.matmul(pt[:], ones_f32[0:1, :], srow[:, n0:n0 + NF])
                nc.scalar.copy(out=bt[:, n0:n0 + NF], in_=pt[:])
            bcast[nm] = bt

        # ---- pass A: P[t, s] = exp(k·q/sqrt(D)); DR rowsums 1 chunk back ----
        rs_ps = [mmp.tile([1, NF], FP32, tag=f"rs{h}", name=f"rs{h}", bufs=1)
                 for h in range(2)]
        chunks = [(ht, r) for ht in range(2) for r in range(G)]

        def emit_rowsum(ci):
            ht, r = chunks[ci]
            jp0 = (r * ST + ht * 4) // 2
            for h in range(2):
                n0 = h * NF
                for jj in range(2):
                    a = 2 * ci + jj
                    nc.tensor.matmul(
                        rs_ps[h][:], ones_dr[:, :, 0:1],
                        P_f8[:, 2 * (jp0 + jj):2 * (jp0 + jj) + 2, n0:n0 + NF],
                        start=(a == 0), stop=(a == 2 * len(chunks) - 1),
                        perf_mode=DR,
                    )

        qtiles = {}

        def issue_qch(ci):
            ht, r = chunks[ci]
            qch = stream.tile([P, DT, NF], FP8, tag="q", name="q", bufs=3)
            nc.sync.dma_start(
                out=qch[:, :, :], in_=ag_q_out[ht][r * P:(r + 1) * P, :]
            )
            qtiles[ci] = qch

        issue_qch(0)
        issue_qch(1)
        for ci, (ht, r) in enumerate(chunks):
            if ci + 2 < len(chunks):
                issue_qch(ci + 2)
            qch = qtiles.pop(ci)
            for tti in range(4):
                j = r * ST + ht * 4 + tti
                for n0 in range(0, S_LOC, NF):
                    ps = mmp.tile([P, NF], FP32, tag="mm", name="mm", bufs=4)
                    for kp in range(KP):
                        nc.tensor.matmul(
                            ps[:], qch[:, 2 * kp:2 * kp + 2, tti * P:(tti + 1) * P],
                            kT_f8[:, 2 * kp:2 * kp + 2, n0:n0 + NF],
                            start=(kp == 0), stop=(kp == KP - 1), perf_mode=DR,
                        )
                    nc.scalar.activation(
                        P_f8[:, j, n0:n0 + NF], ps[:], AF.Exp, scale=EXP_SCALE
                    )
            if ci > 0:
                emit_rowsum(ci - 1)
        emit_rowsum(len(chunks) - 1)

        # raw rowsums -> smalls row 96 (scalar copies: a [1,512] vector op
        # is single-partition and takes ~3.3us); the reciprocal runs after
        # the transpose in [P, 8] form where it's ~100x faster. The tiny
        # transpose is emitted inside pass B (after st0's matmuls) so the
        # PE queue doesn't stall on it before the attention matmuls.
        rs_row = smalls[96:97, :]
        for h in range(2):
            nc.scalar.copy(out=rs_row[0:1, h * NF:(h + 1) * NF], in_=rs_ps[h][:])
        rs8 = const.tile([ST, P], FP32, tag="rs8", name="rs8")
        nc.scalar.dma_start(out=rs8[:, :], in_=rs_row[0:1, :])

        # ---- pass B: attn natural [s, d] + residual -> res (fp32) ----
        # d-half outer: half 0 computes while v's half-1 gather is in flight.
        # gathered-v loads ride the GpSimd queue (idle after the triggers) so
        # the Sync queue's qch stream can't delay them.
        for hh in range(2):
            nc.gpsimd.dma_start(
                out=v_half[hh][:, :, :].rearrange("p (r m) c -> p r (m c)", r=G),
                in_=ag_v_out[hh][:, :].rearrange("(r p) c -> p r c", p=P),
            )

        # LN1 interleaved into the h=1 evictions: stats/apply/cast overlap
        # the next s-tile's attention matmuls; transposes slot into the PE
        # stream.
        x1T_f8 = persist.tile([P, DT, S_LOC], FP8, tag="xT", name="xT")
        h_sh = [persist.tile([P, TJ, NF], FP8, tag=f"vf{h}", name=f"vf{h}")
                for h in range(2)]

        s01 = {}
        for h in range(2):
            n0 = h * NF
            for st in range(ST):
                xre = stage.tile([P, NF], FP32, tag="xre", name="xre")
                nc.scalar.dma_start(
                    out=xre[:], in_=ext["x_ext"][st * P:(st + 1) * P, n0:n0 + NF]
                )
                ps = mmp.tile([P, NF], FP32, tag="mm", name="mm", bufs=4)
                for jp in range(TJ // 2):
                    nc.tensor.matmul(
                        ps[:], P_f8[:, 2 * jp:2 * jp + 2, st * P:(st + 1) * P],
                        v_half[h][:, 2 * jp:2 * jp + 2, :],
                        start=(jp == 0), stop=(jp == TJ // 2 - 1), perf_mode=DR,
                    )
                if h == 0 and st == 0:
                    rt_ps = mmp.tile([P, NF], FP32, tag="mm", name="mm", bufs=4)
                    nc.tensor.transpose(rt_ps[:, 0:ST], rs8[:, :],
                                        ident_f[0:ST, 0:ST])
                    nc.scalar.activation(recipT[:], rt_ps[:, 0:ST], AF.Identity,
                                         scale=SCL)
                    nc.vector.reciprocal(recipT[:], recipT[:])
                if h == 0:
                    sx = stage.tile([P, 2], FP32, tag="s01", name="s01", bufs=8)
                    s01[st] = sx
                    acc = sx[:, 0:1]
                else:
                    acc = s01[st][:, 1:2]
                nc.vector.scalar_tensor_tensor(
                    out=res[st][:, n0:n0 + NF], in0=ps[:], scalar=recipT[:, st:st + 1],
                    in1=xre[:], op0=AluOpType.mult, op1=AluOpType.add,
                    accum_out=acc,
                )
                if h == 1:
                    # LN1: res[st] <- z (normalized); then cast+transpose
                    nm, sd = ln_moments(st, s01[st])
                    nc.vector.tensor_scalar(
                        res[st][:], res[st][:], nm, sd,
                        op0=AluOpType.add, op1=AluOpType.mult,
                    )
                    xb = stage.tile([P, D], BF16, tag="stgb", name="stgb")
                    nc.scalar.copy(out=xb[:], in_=res[st][:])
                    transpose_to(mmp, xb, x1T_f8, st * P)

        # ---- FFN1: stream W1 once (group-outer, s-half-inner) ----
        for g in range(HG):
            w1g = (w1g3r if g == HG - 1
                   else load_w8(ext["w18_ext"], g * KP * P))
            for sh in range(2):
                n0 = sh * NF
                for mh_i in range(HPG):
                    mh = g * HPG + mh_i
                    pt = mmp.tile([P, NF], FP32, tag="mm", name="mm", bufs=4)
                    for kp in range(KP):
                        nc.tensor.matmul(
                            pt[:], w1g[kp][:, :, mh_i * P:(mh_i + 1) * P],
                            x1T_f8[:, 2 * kp:2 * kp + 2, n0:n0 + NF],
                            start=(kp == 0), stop=(kp == KP - 1), perf_mode=DR,
                        )
                    nc.scalar.activation(
                        h_sh[sh][:, mh, :], pt[:], AF.Gelu,
                        bias=b1_sb[mh], scale=1.0 / SCL,
                    )

    # ================= phase B: FFN2 (fp8 DR) + LN2 + out =================
    # 2 passes of 4 s-tiles (8 PSUM banks). Chains START on the resident
    # W2 tail tiles, then consume streamed k 0..12; per-s-tile chain ends
    # stagger on k=12 so the epilogues pipeline while pass 2's resident
    # matmuls keep the PE warm.
    with tcx.tile_pool(name="psB", bufs=1, space="PSUM") as f2p:
        for sp, sts in enumerate([(0, 1, 2, 3), (4, 5, 6, 7)]):
            f2 = {(st, h): f2p.tile([P, NF], FP32, tag=f"f{st % 4}_{h}",
                                    name=f"f{st % 4}_{h}")
                  for st in sts for h in range(2)}

            def f2mm(kp2, st, h, w2t, start=False, stop=False):
                nc.tensor.matmul(
                    f2[(st, h)][:],
                    h_sh[st // 4][:, 2 * kp2:2 * kp2 + 2,
                                  (st % 4) * P:(st % 4 + 1) * P],
                    w2t[:, :, h * NF:(h + 1) * NF],
                    start=start, stop=stop, perf_mode=DR,
                )

            # resident tail first: starts the accumulation groups
            for st in sts:
                for h in range(2):
                    for i in range(W2R):
                        f2mm(W2S + i, st, h, w2r[i], start=(i == 0))

            # streamed k-pairs 0..W2S-1; stagger chain ends on the last one
            for kp2 in range(W2S):
                wt = stream.tile([P, 2, D], FP8, tag=f"w{kp2 % KP}",
                                 name=f"w{kp2 % KP}")
                nc.sync.dma_start(
                    out=wt[:, :, :],
                    in_=ext["w28_ext"][kp2 * P:(kp2 + 1) * P, :],
                )
                if kp2 == 0 and trivial_gb:
                    # pre-add beta+b2 into res here (vector is idle during
                    # the matmul stream) so the per-tile tail epilogue shrinks
                    for st in sts:
                        nc.vector.tensor_add(res[st][:], res[st][:],
                                             bcast["beta_b2"][:])
                if kp2 < W2S - 1:
                    for st in sts:
                        for h in range(2):
                            f2mm(kp2, st, h, wt)
                else:
                    for st in sts:
                        for h in range(2):
                            f2mm(kp2, st, h, wt, stop=True)
                        # epilogue streams per s-tile as its chains end
                        s2 = stage.tile([P, 2], FP32, tag="s01", name="s01",
                                        bufs=8)
                        if trivial_gb:
                            # pre-LN2 = z + (beta+b2) + f2/SCL2 (beta_b2
                            # pre-added above)
                            for h in range(2):
                                n0 = h * NF
                                nc.vector.scalar_tensor_tensor(
                                    out=res[st][:, n0:n0 + NF], in0=f2[(st, h)][:],
                                    scalar=1.0 / SCL2, in1=res[st][:, n0:n0 + NF],
                                    op0=AluOpType.mult, op1=AluOpType.add,
                                    accum_out=s2[:, h:h + 1],
                                )
                        else:
                            t2 = stage.tile([P, D], FP32, tag="stgf2", name="stgf2")
                            nc.vector.tensor_mul(t2[:], res[st][:], bcast["gamma"][:])
                            for h in range(2):
                                n0 = h * NF
                                nc.vector.scalar_tensor_tensor(
                                    out=t2[:, n0:n0 + NF], in0=f2[(st, h)][:],
                                    scalar=1.0 / SCL2, in1=t2[:, n0:n0 + NF],
                                    op0=AluOpType.mult, op1=AluOpType.add,
                                )
                            nc.vector.tensor_add(res[st][:], t2[:],
                                                 bcast["beta_b2"][:])
                            nc.vector.reduce_sum(s2[:, 0:1], res[st][:, 0:NF])
                            nc.vector.reduce_sum(s2[:, 1:2], res[st][:, NF:D])
                        # LN2 + store
                        nm, sd = ln_moments(st, s2)
                        ot = stage.tile([P, D], FP32, tag="stgf", name="stgf",
                                        bufs=3)
                        nc.vector.tensor_scalar(
                            ot[:], res[st][:], nm, sd,
                            op0=AluOpType.add, op1=AluOpType.mult,
                        )
                        if not trivial_gb:
                            nc.vector.tensor_mul(ot[:], ot[:], bcast["gamma"][:])
                            nc.vector.tensor_add(ot[:], ot[:], bcast["beta"][:])
                        nc.gpsimd.dma_start(
                            out=ext["out_ext"][st * P:(st + 1) * P, :], in_=ot[:]
                        )


def build_nc(trivial_gb):
    nc = bacc.Bacc(target_bir_lowering=False, num_devices=N_CORES)

    ext = {
        "x_ext": nc.declare_dram_parameter("x", [S_LOC, D], FP32, isOutput=False),
        "wq8_ext": nc.declare_dram_parameter("wq8", [KP * P, 2 * D], FP8, isOutput=False),
        "wk8_ext": nc.declare_dram_parameter("wk8", [KP * P, 2 * D], FP8, isOutput=False),
        "wv8_ext": nc.declare_dram_parameter("wv8", [KP * P, 2 * D], FP8, isOutput=False),
        "w18_ext": nc.declare_dram_parameter("w18", [HG * KP * P, 2 * D], FP8, isOutput=False),
        "w28_ext": nc.declare_dram_parameter("w28", [(HT // 2) * P, 2 * D], FP8, isOutput=False),
        "bqp_ext": nc.declare_dram_parameter("bqp", [P, DT], FP32, isOutput=False),
        "bkp_ext": nc.declare_dram_parameter("bkp", [P, DT], FP32, isOutput=False),
        "b1p_ext": nc.declare_dram_parameter("b1p", [P, HT], FP32, isOutput=False),
        "bv_ext": nc.declare_dram_parameter("bv", [1, D], FP32, isOutput=False),
        "beta_b2_ext": nc.declare_dram_parameter("beta_b2", [1, D], FP32, isOutput=False),
        "gamma_ext": nc.declare_dram_parameter("gamma", [1, D], FP32, isOutput=False),
        "beta_ext": nc.declare_dram_parameter("beta", [1, D], FP32, isOutput=False),
        "out_ext": nc.declare_dram_parameter("out", [S_LOC, D], FP32, isOutput=True),
    }

    with tile.TileContext(nc) as tc:
        with (
            tc.tile_pool(name="dram", bufs=1, space="DRAM") as dram,
            tc.tile_pool(name="const", bufs=1) as const,
            tc.tile_pool(name="persist", bufs=1) as persist,
            tc.tile_pool(name="stage", bufs=2) as stage,
            tc.tile_pool(name="stream", bufs=2) as stream,
        ):
            ext.update(tc=tc, dram=dram, const=const, persist=persist,
                       stage=stage, stream=stream)
            build_graph(nc, tc, ext, trivial_gb)
    nc.compile()
    return nc


_NC_CACHE = {}


def _get_nc(trivial_gb):
    if trivial_gb not in _NC_CACHE:
        _NC_CACHE[trivial_gb] = build_nc(trivial_gb)
    return _NC_CACHE[trivial_gb]


F8NP = ml_dtypes.float8_e4m3


def _pair_rows(w):
    # [K, N] -> pair layout: rows kp*128+p, cols i*N+c = w[(2kp+i)*128+p, c]
    k, n = w.shape
    kp = k // (2 * P)
    w4 = w.reshape(kp, 2, P, n).transpose(0, 2, 1, 3).reshape(kp * P, 2 * n)
    return np.ascontiguousarray(w4)


def _col_pack(v, n):
    # [n*128] -> [128, n] with out[p, m] = v[m*128 + p]
    return np.ascontiguousarray(v.reshape(n, P).T)


def _make_in_maps(inputs):
    x = np.asarray(inputs["input_embedding"], dtype=np.float32)
    assert x.shape == (B, S, D), x.shape

    gamma = np.asarray(inputs["gamma"], np.float32).reshape(D)
    beta = np.asarray(inputs["beta"], np.float32).reshape(D)
    trivial_gb = bool(np.all(gamma == 1.0) and np.all(beta == 0.0))
    W1 = np.asarray(inputs["W1"], np.float32)
    b1 = np.asarray(inputs["b1"], np.float32).reshape(H)
    # fold LN1's gamma/beta into W1/b1 (FFN1 consumes the normalized z)
    W1f = gamma[:, None] * W1
    b1f = b1 + beta @ W1
    # W1 group-major pair layout: rows (g*KP+kp)*128+p, cols i*D+c
    w1g = (SCL * W1f).reshape(KP, 2, P, HG, D).transpose(3, 0, 2, 1, 4)
    w18 = np.ascontiguousarray(w1g.reshape(HG * KP * P, 2 * D)).astype(F8NP)

    shared = {
        "wq8": _pair_rows(SCL * np.asarray(inputs["Wq"], np.float32)).astype(F8NP),
        "wk8": _pair_rows(SCL * np.asarray(inputs["Wk"], np.float32)).astype(F8NP),
        "wv8": _pair_rows(SCL * np.asarray(inputs["Wv"], np.float32)).astype(F8NP),
        "w18": w18,
        "w28": _pair_rows(SCL2 * np.asarray(inputs["W2"], np.float32)).astype(F8NP),
        "bqp": _col_pack(SCL * np.asarray(inputs["bq"], np.float32).reshape(D), DT),
        "bkp": _col_pack(SCL * np.asarray(inputs["bk"], np.float32).reshape(D), DT),
        "b1p": _col_pack(b1f, HT),
        "bv": SCL * np.asarray(inputs["bv"], np.float32).reshape(1, D),
        "beta_b2": (beta + np.asarray(inputs["b2"], np.float32).reshape(D)).reshape(1, D),
        "gamma": gamma.reshape(1, D),
        "beta": beta.reshape(1, D),
    }

    in_maps = []
    for c in range(N_CORES):
        b = c // G
        r = c % G
        m = dict(shared)
        m["x"] = np.ascontiguousarray(x[b, r * S_LOC:(r + 1) * S_LOC, :])
        in_maps.append(m)
    return in_maps, trivial_gb


def kernel(**inputs: np.ndarray) -> np.ndarray:
    from concourse.bass_utils import run_bass_kernel_spmd

    in_maps, trivial_gb = _make_in_maps(inputs)
    nc = _get_nc(trivial_gb)
    res = run_bass_kernel_spmd(nc, in_maps, core_ids=list(range(N_CORES)))

    out = np.empty((B, S, D), dtype=np.float32)
    for c in range(N_CORES):
        b = c // G
        r = c % G
        out[b, r * S_LOC:(r + 1) * S_LOC, :] = res.results[c]["out"]
    return out


# revision 43
# speedup vs baseline: 1.1474x; 1.0149x over previous
"""Distributed Trainium2 kernel for a transformer attention block (B=2, S=4096,
D=1024, H=4096, fp32 I/O).

Reference computation (note the Q<-k, K<-q, V<-v argument quirk):
    k = x @ Wk + bk ; q = x @ Wq + bq ; v = x @ Wv + bv
    scores[s,t] = k[s]·q[t] / sqrt(D); attn = softmax_t(scores) @ v
    x1 = LN(x + attn); h = gelu(x1 @ W1 + b1); out = LN(x1 + h @ W2 + b2)

Sharding: 8 cores -> 2 groups of 4 (one group per batch element); each core
owns 1024 sequence rows. Design notes:
  - all five GEMMs run fp8 (e4m3) DoubleRow matmuls. Weights are pre-cast/
    pre-tiled on the host into the DoubleRow pair layout. Host pre-scales
    Wq/Wk/Wv/W1 by 32 and W2 by 64; inverse scales fold into activation
    scale constants.
  - a leading dummy AllGather absorbs program-launch skew so the real q/v
    AllGathers handshake fast. All gather-input stores and gathered-output
    loads are issued from the GpSimd queue so the Sync queue (x/weight
    loads) never blocks behind a collective wait, and the collective
    triggers sequence naturally behind their input stores.
  - LayerNorms run as moment sums: the PSUM->SBUF evictions carry
    accum_out partials (sum x), a scalar-engine Square pass accumulates
    sum x^2, and the tiny per-partition algebra runs on vector. LN1 is
    interleaved into pass B's second half so its transposes slot into the
    PE stream while vector/scalar overlap the attention matmuls.
  - FFN1 streams W1 once (group-outer, s-half-inner). FFN2 runs 2 passes
    of 4 s-tiles (all 8 PSUM banks); W2 k-tiles 13..15 stay resident so
    each pass's chains START on them and the per-s-tile chain ends stagger
    on the last streamed tile, letting epilogues drain while the next
    pass's resident matmuls keep the PE warm.
"""

import sys

if "/opt/trn_rl_repo" not in sys.path:
    sys.path.insert(0, "/opt/trn_rl_repo")

import numpy as np
import ml_dtypes

import concourse.bacc as bacc
import concourse.mybir as mybir
import concourse.tile as tile
from concourse.alu_op_type import AluOpType
from concourse.masks import make_identity


AF = mybir.ActivationFunctionType
FP32 = mybir.dt.float32
BF16 = mybir.dt.bfloat16
FP8 = mybir.dt.float8e4
DR = mybir.MatmulPerfMode.DoubleRow

B, S, D, H = 2, 4096, 1024, 4096
N_CORES = 8
G = 4                 # cores per group (one group per batch element)
S_LOC = S // G        # sequence rows per core
P = 128               # SBUF partitions
NF = 512              # matmul moving free-dim (one fp32 PSUM bank)
DT = D // P           # 8 d-tiles
KP = DT // 2          # 4 k-subtile pairs over D
ST = S_LOC // P       # 8 s-tiles per core
TJ = S // P           # 32 global t-subtiles
HT = H // P           # 32 h-tiles
HG = 4                # FFN1 weight-streaming groups
HPG = HT // HG        # 8 h-tiles per group
W2R = 3               # resident W2 tail k-pair tiles (13, 14, 15)
W2S = HT // 2 - W2R   # streamed W2 k-pair tiles per pass (0..12)
EPS = 1e-5
SCL = 32.0            # host pre-scale on Wq/Wk/Wv/W1
SCL2 = 64.0           # host pre-scale on W2
SM_SCALE = 1.0 / float(np.sqrt(np.float32(D)))
EXP_SCALE = SM_SCALE / (SCL * SCL)

GROUPS = [[0, 1, 2, 3], [4, 5, 6, 7]]


def build_graph(nc, tc, ext, trivial_gb):
    stream = ext["stream"]
    persist = ext["persist"]
    stage = ext["stage"]
    const = ext["const"]
    dram = ext["dram"]
    tcx = ext["tc"]

    # ---- constants ----
    ident_bf = const.tile([P, P], BF16, tag="ident_bf", name="ident_bf")
    make_identity(nc, ident_bf[:])
    ident_f = const.tile([P, P], FP32, tag="ident_f", name="ident_f")
    make_identity(nc, ident_f[:])
    ones_dr = const.tile([P, 2, 16], FP8, tag="ones_dr", name="ones_dr")
    nc.vector.memset(ones_dr[:, :, :], 1.0)
    ones_f32 = const.tile([1, P], FP32, tag="ones_f32", name="ones_f32")
    nc.vector.memset(ones_f32[:], 1.0)
    eps_t = const.tile([P, 1], FP32, tag="eps", name="eps")
    nc.vector.memset(eps_t[:], EPS)

    # biases arrive host-packed: [P, 8] bq | [P, 8] bk | [P, 32] b1.
    # (Their DMAs are issued after the first x loads so the cold-start x
    # chunk leads the Sync queue.)
    pvecs = const.tile([P, 48], FP32, tag="pvecs", name="pvecs")
    bq_sb = [pvecs[:, m:m + 1] for m in range(DT)]
    bk_sb = [pvecs[:, DT + m:DT + m + 1] for m in range(DT)]
    b1_sb = [pvecs[:, 2 * DT + m:2 * DT + m + 1] for m in range(HT)]

    # free-dim [1, D] rows at 32-aligned partitions (matmul-legal bases);
    # row 96 is scratch for the softmax recip row (never a matmul operand)
    smalls = const.tile([P, D], FP32, tag="smalls", name="smalls")
    SROW = {"bv": 0, "gamma": 32, "beta": 64}
    smalls2 = const.tile([1, D], FP32, tag="smalls2", name="smalls2")

    def load_small_consts():
        nc.sync.dma_start(out=pvecs[:, 0:DT], in_=ext["bqp_ext"][:, :])
        nc.sync.dma_start(out=pvecs[:, DT:2 * DT], in_=ext["bkp_ext"][:, :])
        nc.sync.dma_start(out=pvecs[:, 2 * DT:2 * DT + HT], in_=ext["b1p_ext"][:, :])
        for nm, r in SROW.items():
            nc.sync.dma_start(out=smalls[r:r + 1, :], in_=ext[nm + "_ext"][0:1, :])
        nc.sync.dma_start(out=smalls2[0:1, :], in_=ext["beta_b2_ext"][0:1, :])

    res = [persist.tile([P, D], FP32, tag=f"res{m}", name=f"res{m}") for m in range(ST)]
    xT_f8 = persist.tile([P, DT, S_LOC], FP8, tag="xT", name="xT")
    qT_f8 = persist.tile([P, DT, S_LOC], FP8, tag="qT", name="qT")
    kT_f8 = persist.tile([P, DT, S_LOC], FP8, tag="kT", name="kT")
    v_half = [persist.tile([P, TJ, NF], FP8, tag=f"vf{h}", name=f"vf{h}")
              for h in range(2)]
    P_f8 = persist.tile([P, TJ, S_LOC], FP8, tag="pf", name="pf")
    # resident W2 tail k-pair tiles (loaded during phase A, used by both
    # FFN2 passes), and FFN1's last weight group (its streamed slot would
    # WAR-stall against group 0's matmuls right at the FFN1->FFN2 seam)
    w2r = [persist.tile([P, 2, D], FP8, tag=f"w2r{i}", name=f"w2r{i}")
           for i in range(W2R)]
    w1g3r = [persist.tile([P, 2, D], FP8, tag=f"w1g3r{i}", name=f"w1g3r{i}")
             for i in range(KP)]

    # gather buffers are partition-major ([p, tile, col] flattened) so the
    # post-gather loads are single DMAs with 4KB-contiguous runs
    ag_q_in = [dram.tile([P, DT * NF], FP8, name=f"agqi{h}") for h in range(2)]
    ag_q_out = [dram.tile([G * P, DT * NF], FP8, name=f"agqo{h}") for h in range(2)]
    ag_v_in = [dram.tile([P, ST * NF], FP8, name=f"agvi{h}") for h in range(2)]
    ag_v_out = [dram.tile([G * P, ST * NF], FP8, name=f"agvo{h}") for h in range(2)]

    bcast = {}
    recipT = const.tile([P, ST], FP32, tag="recipT", name="recipT")

    def load_w8(ext_t, base_row, eng=None):
        eng = eng or nc.sync
        tiles = []
        for kp in range(KP):
            wt = stream.tile([P, 2, D], FP8, tag=f"w{kp}", name=f"w{kp}")
            r0 = base_row + kp * P
            eng.dma_start(out=wt[:, :, :], in_=ext_t[r0:r0 + P, :])
            tiles.append(wt)
        return tiles

    # LayerNorm via moments: sums arrive via accum_out on the evictions
    # (s01), sum-of-squares via a scalar-engine Square pass. Returns
    # (negmu*sd, sd) for the fused apply  (x + nm/sd)*sd = (x-mu)*sd.
    def ln_moments(st, s01):
        lt = stage.tile([P, 8], FP32, tag="lnt", name="lnt", bufs=3)
        sq = stage.tile([P, D], FP32, tag="sqt", name="sqt", bufs=1)
        sumsq = lt[:, 0:1]
        nc.scalar.activation(sq[:], res[st][:], AF.Square, accum_out=sumsq)
        ssum = lt[:, 1:2]
        nc.vector.tensor_add(ssum, s01[:, 0:1], s01[:, 1:2])
        negmu = lt[:, 2:3]
        nc.vector.tensor_scalar_mul(negmu, ssum, -1.0 / D)
        var = lt[:, 3:4]
        mu2 = lt[:, 4:5]
        nc.vector.tensor_mul(mu2, negmu, negmu)
        nc.vector.scalar_tensor_tensor(
            out=var, in0=sumsq, scalar=1.0 / D, in1=mu2,
            op0=AluOpType.mult, op1=AluOpType.subtract,
        )
        sd = lt[:, 5:6]
        nc.scalar.activation(sd, var, AF.Sqrt, bias=eps_t[:])
        nc.vector.reciprocal(sd, sd)
        nm = lt[:, 6:7]
        nc.vector.tensor_mul(nm, negmu, sd)
        return nm, sd

    def transpose_to(mmp, src_bf, dst_f8, s0):
        tp = mmp.tile([P, DT * P], BF16, tag="trp", name="trp", bufs=2)
        for dj in range(DT):
            nc.tensor.transpose(
                tp[:, dj * P:(dj + 1) * P], src_bf[:, dj * P:(dj + 1) * P],
                ident_bf[:],
            )
        nc.vector.tensor_copy(
            out=dst_f8[:, :, s0:s0 + P],
            in_=tp[:].rearrange("p (d s) -> p d s", d=DT),
        )

    # ================= phase A: QKV, attention, LN1, FFN1 =================
    with tcx.tile_pool(name="psA", bufs=1, space="PSUM") as mmp:
        # PE warm-up: ~40 dependency-free matmuls while the first x DMA is
        # in flight. The HAM clock gate needs ~3.4us of sustained PE
        # activity before it lifts the 1.2GHz cold throttle — burning that
        # window on throwaway work makes the real QKV matmuls run at 2.4GHz.
        for _ in range(40):
            wp = mmp.tile([P, NF], FP32, tag="mm", name="mm", bufs=4)
            nc.tensor.matmul(wp[:, 0:P], ident_bf[:], ident_bf[:])
        # ---- x -> xT fp8: first s-half, then q-half0 can go ----
        # loads alternate between the Sync and Scalar queues so the 4MB of
        # x doesn't trickle through a single DMA ring at cold start
        x_eng = [nc.sync, nc.scalar, nc.sync, nc.scalar]

        def load_x_half(h):
            for si in range(h * 4, h * 4 + 4):
                xn = stage.tile([P, D], FP32, tag="stgf", name="stgf", bufs=3)
                xb = stage.tile([P, D], BF16, tag="stgb", name="stgb")
                eng = x_eng[si % 4]
                if si == 0:
                    # split the first tile so the cold-start PE work begins
                    # after ~256KB instead of ~512KB of DMA
                    for c in range(2):
                        eng.dma_start(
                            out=xn[:, c * NF:(c + 1) * NF],
                            in_=ext["x_ext"][si * P:(si + 1) * P, c * NF:(c + 1) * NF],
                        )
                        nc.vector.tensor_copy(
                            out=xb[:, c * NF:(c + 1) * NF],
                            in_=xn[:, c * NF:(c + 1) * NF],
                        )
                else:
                    eng.dma_start(out=xn[:], in_=ext["x_ext"][si * P:(si + 1) * P, :])
                    nc.vector.tensor_copy(out=xb[:], in_=xn[:])
                transpose_to(mmp, xb, xT_f8, si * P)

        def q_half(h):
            n0 = h * NF
            for m in range(DT):
                pt = mmp.tile([P, NF], FP32, tag="mm", name="mm", bufs=4)
                for kp in range(KP):
                    nc.tensor.matmul(
                        pt[:], wq[kp][:, :, m * P:(m + 1) * P],
                        xT_f8[:, 2 * kp:2 * kp + 2, n0:n0 + NF],
                        start=(kp == 0), stop=(kp == KP - 1), perf_mode=DR,
                    )
                nc.scalar.activation(qT_f8[:, m, n0:n0 + NF], pt[:], AF.Identity,
                                     bias=bq_sb[m])
                nc.gpsimd.dma_start(
                    out=ag_q_in[h][:, m * NF:(m + 1) * NF],
                    in_=qT_f8[:, m, n0:n0 + NF],
                )
            nc.gpsimd.collective_compute(
                "AllGather", AluOpType.bypass, replica_groups=GROUPS,
                ins=[ag_q_in[h][:].opt()], outs=[ag_q_out[h][:].opt()],
            )

        wq = load_w8(ext["wq8_ext"], 0, eng=nc.scalar)
        load_x_half(0)
        load_small_consts()
        q_half(0)
        load_x_half(1)
        q_half(1)

        # ---- v = x @ (32 Wv) + 32 bv (natural, fp8); AllGather ----
        wv = load_w8(ext["wv8_ext"], 0)
        bv_b = const.tile([P, D], FP32, tag="bc_bv", name="bc_bv")
        for n0 in range(0, D, NF):
            pt = mmp.tile([P, NF], FP32, tag="mm", name="mm", bufs=4)
            nc.tensor.matmul(pt[:], ones_f32[0:1, :], smalls[0:1, n0:n0 + NF])
            nc.scalar.copy(out=bv_b[:, n0:n0 + NF], in_=pt[:])
        for mt in range(ST):
            v8 = stage.tile([P, D], FP8, tag="v8", name="v8")
            for n0 in range(0, D, NF):
                pt = mmp.tile([P, NF], FP32, tag="mm", name="mm", bufs=4)
                for kp in range(KP):
                    nc.tensor.matmul(
                        pt[:], xT_f8[:, 2 * kp:2 * kp + 2, mt * P:(mt + 1) * P],
                        wv[kp][:, :, n0:n0 + NF],
                        start=(kp == 0), stop=(kp == KP - 1), perf_mode=DR,
                    )
                nc.vector.tensor_add(
                    v8[:, n0:n0 + NF], pt[:], bv_b[:, n0:n0 + NF]
                )
            for hh in range(2):
                nc.gpsimd.dma_start(
                    out=ag_v_in[hh][:, mt * NF:(mt + 1) * NF],
                    in_=v8[:, hh * NF:(hh + 1) * NF],
                )
        # v gathered in two d-halves so pass B's first half can start while
        # the second half is still on the wire (the CC stream is serial)
        for hh in range(2):
            nc.gpsimd.collective_compute(
                "AllGather", AluOpType.bypass, replica_groups=GROUPS,
                ins=[ag_v_in[hh][:].opt()], outs=[ag_v_out[hh][:].opt()],
            )

        # ---- kT = (32 Wk).T @ x + 32 bk (fp8, local) ----
        wk = load_w8(ext["wk8_ext"], 0)
        # resident W2 tail tiles + FFN1's last weight group: load now on the
        # Scalar queue (no sem waits there) so the Sync queue's qch stream
        # leads right at the q0-mesh end (consumed in FFN1/FFN2)
        for i in range(W2R):
            kp2 = W2S + i
            nc.scalar.dma_start(
                out=w2r[i][:, :, :], in_=ext["w28_ext"][kp2 * P:(kp2 + 1) * P, :]
            )
        for kp in range(KP):
            r0 = ((HG - 1) * KP + kp) * P
            nc.scalar.dma_start(
                out=w1g3r[kp][:, :, :], in_=ext["w18_ext"][r0:r0 + P, :]
            )
        for m in range(DT):
            for n0 in range(0, S_LOC, NF):
                pt = mmp.tile([P, NF], FP32, tag="mm", name="mm", bufs=4)
                for kp in range(KP):
                    nc.tensor.matmul(
                        pt[:], wk[kp][:, :, m * P:(m + 1) * P],
                        xT_f8[:, 2 * kp:2 * kp + 2, n0:n0 + NF],
                        start=(kp == 0), stop=(kp == KP - 1), perf_mode=DR,
                    )
                nc.scalar.activation(kT_f8[:, m, n0:n0 + NF], pt[:], AF.Identity,
                                     bias=bk_sb[m])

        # [P, D] broadcasts, off the critical path (fills AG wait)
        bc_rows = [("gamma", smalls[32:33, :]),
                   ("beta", smalls[64:65, :]),
                   ("beta_b2", smalls2[0:1, :])]
        if trivial_gb:
            bc_rows = [bc_rows[2]]  # only beta+b2 needed
        for nm, srow in bc_rows:
            bt = const.tile([P, D], FP32, tag=f"bc_{nm}", name=f"bc_{nm}")
            for n0 in range(0, D, NF):
                pt = mmp.tile([P, NF], FP32, tag="mm", name="mm", bufs=4)
                nc.tensor.matmul(pt[:], ones_f32[0:1, :], srow[:, n0:n0 + NF])
                nc.scalar.copy(out=bt[:, n0:n0 + NF], in_=pt[:])
            bcast[nm] = bt

        # ---- pass A: P[t, s] = exp(k·q/sqrt(D)); DR rowsums 1 chunk back ----
        rs_ps = [mmp.tile([1, NF], FP32, tag=f"rs{h}", name=f"rs{h}", bufs=1)
                 for h in range(2)]
        chunks = [(ht, r) for ht in range(2) for r in range(G)]

        def emit_rowsum(ci):
            ht, r = chunks[ci]
            jp0 = (r * ST + ht * 4) // 2
            for h in range(2):
                n0 = h * NF
                for jj in range(2):
                    a = 2 * ci + jj
                    nc.tensor.matmul(
                        rs_ps[h][:], ones_dr[:, :, 0:1],
                        P_f8[:, 2 * (jp0 + jj):2 * (jp0 + jj) + 2, n0:n0 + NF],
                        start=(a == 0), stop=(a == 2 * len(chunks) - 1),
                        perf_mode=DR,
                    )

        qtiles = {}

        def issue_qch(ci):
            ht, r = chunks[ci]
            qch = stream.tile([P, DT, NF], FP8, tag="q", name="q", bufs=3)
            nc.sync.dma_start(
                out=qch[:, :, :], in_=ag_q_out[ht][r * P:(r + 1) * P, :]
            )
            qtiles[ci] = qch

        issue_qch(0)
        issue_qch(1)
        for ci, (ht, r) in enumerate(chunks):
            if ci + 2 < len(chunks):
                issue_qch(ci + 2)
            qch = qtiles.pop(ci)
            for tti in range(4):
                j = r * ST + ht * 4 + tti
                for n0 in range(0, S_LOC, NF):
                    ps = mmp.tile([P, NF], FP32, tag="mm", name="mm", bufs=4)
                    for kp in range(KP):
                        nc.tensor.matmul(
                            ps[:], qch[:, 2 * kp:2 * kp + 2, tti * P:(tti + 1) * P],
                            kT_f8[:, 2 * kp:2 * kp + 2, n0:n0 + NF],
                            start=(kp == 0), stop=(kp == KP - 1), perf_mode=DR,
                        )
                    nc.scalar.activation(
                        P_f8[:, j, n0:n0 + NF], ps[:], AF.Exp, scale=EXP_SCALE
                    )
            if ci > 0:
                emit_rowsum(ci - 1)
        emit_rowsum(len(chunks) - 1)

        # raw rowsums -> smalls row 96 (scalar copies: a [1,512] vector op
        # is single-partition and takes ~3.3us); the reciprocal runs after
        # the transpose in [P, 8] form where it's ~100x faster. The tiny
        # transpose is emitted inside pass B (after st0's matmuls) so the
        # PE queue doesn't stall on it before the attention matmuls.
        rs_row = smalls[96:97, :]
        for h in range(2):
            nc.scalar.copy(out=rs_row[0:1, h * NF:(h + 1) * NF], in_=rs_ps[h][:])
        rs8 = const.tile([ST, P], FP32, tag="rs8", name="rs8")
        nc.scalar.dma_start(out=rs8[:, :], in_=rs_row[0:1, :])

        # ---- pass B: attn natural [s, d] + residual -> res (fp32) ----
        # d-half outer: half 0 computes while v's half-1 gather is in flight.
        # gathered-v loads ride the GpSimd queue (idle after the triggers) so
        # the Sync queue's qch stream can't delay them.
        for hh in range(2):
            nc.gpsimd.dma_start(
                out=v_half[hh][:, :, :].rearrange("p (r m) c -> p r (m c)", r=G),
                in_=ag_v_out[hh][:, :].rearrange("(r p) c -> p r c", p=P),
            )

        # LN1 interleaved into the h=1 evictions: stats/apply/cast overlap
        # the next s-tile's attention matmuls; transposes slot into the PE
        # stream.
        x1T_f8 = persist.tile([P, DT, S_LOC], FP8, tag="xT", name="xT")
        h_sh = [persist.tile([P, TJ, NF], FP8, tag=f"vf{h}", name=f"vf{h}")
                for h in range(2)]

        s01 = {}
        for h in range(2):
            n0 = h * NF
            for st in range(ST):
                xre = stage.tile([P, NF], FP32, tag="xre", name="xre")
                nc.scalar.dma_start(
                    out=xre[:], in_=ext["x_ext"][st * P:(st + 1) * P, n0:n0 + NF]
                )
                ps = mmp.tile([P, NF], FP32, tag="mm", name="mm", bufs=4)
                for jp in range(TJ // 2):
                    nc.tensor.matmul(
                        ps[:], P_f8[:, 2 * jp:2 * jp + 2, st * P:(st + 1) * P],
                        v_half[h][:, 2 * jp:2 * jp + 2, :],
                        start=(jp == 0), stop=(jp == TJ // 2 - 1), perf_mode=DR,
                    )
                if h == 0 and st == 0:
                    rt_ps = mmp.tile([P, NF], FP32, tag="mm", name="mm", bufs=4)
                    nc.tensor.transpose(rt_ps[:, 0:ST], rs8[:, :],
                                        ident_f[0:ST, 0:ST])
                    nc.scalar.activation(recipT[:], rt_ps[:, 0:ST], AF.Identity,
                                         scale=SCL)
                    nc.vector.reciprocal(recipT[:], recipT[:])
                if h == 0:
                    sx = stage.tile([P, 2], FP32, tag="s01", name="s01", bufs=8)
                    s01[st] = sx
                    acc = sx[:, 0:1]
                else:
                    acc = s01[st][:, 1:2]
                nc.vector.scalar_tensor_tensor(
                    out=res[st][:, n0:n0 + NF], in0=ps[:], scalar=recipT[:, st:st + 1],
                    in1=xre[:], op0=AluOpType.mult, op1=AluOpType.add,
                    accum_out=acc,
                )
                if h == 1:
                    # LN1: res[st] <- z (normalized); then cast+transpose
                    nm, sd = ln_moments(st, s01[st])
                    nc.vector.tensor_scalar(
                        res[st][:], res[st][:], nm, sd,
                        op0=AluOpType.add, op1=AluOpType.mult,
                    )
                    xb = stage.tile([P, D], BF16, tag="stgb", name="stgb")
                    nc.scalar.copy(out=xb[:], in_=res[st][:])
                    transpose_to(mmp, xb, x1T_f8, st * P)

        # ---- FFN1: stream W1 once (group-outer, s-half-inner) ----
        for g in range(HG):
            w1g = (w1g3r if g == HG - 1
                   else load_w8(ext["w18_ext"], g * KP * P))
            for sh in range(2):
                n0 = sh * NF
                for mh_i in range(HPG):
                    mh = g * HPG + mh_i
                    pt = mmp.tile([P, NF], FP32, tag="mm", name="mm", bufs=4)
                    for kp in range(KP):
                        nc.tensor.matmul(
                            pt[:], w1g[kp][:, :, mh_i * P:(mh_i + 1) * P],
                            x1T_f8[:, 2 * kp:2 * kp + 2, n0:n0 + NF],
                            start=(kp == 0), stop=(kp == KP - 1), perf_mode=DR,
                        )
                    nc.scalar.activation(
                        h_sh[sh][:, mh, :], pt[:], AF.Gelu,
                        bias=b1_sb[mh], scale=1.0 / SCL,
                    )

    # ================= phase B: FFN2 (fp8 DR) + LN2 + out =================
    # 2 passes of 4 s-tiles (8 PSUM banks). Chains START on the resident
    # W2 tail tiles, then consume streamed k 0..12; per-s-tile chain ends
    # stagger on k=12 so the epilogues pipeline while pass 2's resident
    # matmuls keep the PE warm.
    with tcx.tile_pool(name="psB", bufs=1, space="PSUM") as f2p:
        for sp, sts in enumerate([(0, 1, 2, 3), (4, 5, 6, 7)]):
            f2 = {(st, h): f2p.tile([P, NF], FP32, tag=f"f{st % 4}_{h}",
                                    name=f"f{st % 4}_{h}")
                  for st in sts for h in range(2)}

            def f2mm(kp2, st, h, w2t, start=False, stop=False):
                nc.tensor.matmul(
                    f2[(st, h)][:],
                    h_sh[st // 4][:, 2 * kp2:2 * kp2 + 2,
                                  (st % 4) * P:(st % 4 + 1) * P],
                    w2t[:, :, h * NF:(h + 1) * NF],
                    start=start, stop=stop, perf_mode=DR,
                )

            # resident tail first: starts the accumulation groups
            for st in sts:
                for h in range(2):
                    for i in range(W2R):
                        f2mm(W2S + i, st, h, w2r[i], start=(i == 0))

            # streamed k-pairs 0..W2S-1; stagger chain ends on the last one
            for kp2 in range(W2S):
                wt = stream.tile([P, 2, D], FP8, tag=f"w{kp2 % KP}",
                                 name=f"w{kp2 % KP}")
                nc.sync.dma_start(
                    out=wt[:, :, :],
                    in_=ext["w28_ext"][kp2 * P:(kp2 + 1) * P, :],
                )
                if kp2 == 0 and trivial_gb:
                    # pre-add beta+b2 into res here (vector is idle during
                    # the matmul stream) so the per-tile tail epilogue shrinks
                    for st in sts:
                        nc.vector.tensor_add(res[st][:], res[st][:],
                                             bcast["beta_b2"][:])
                if kp2 < W2S - 1:
                    for st in sts:
                        for h in range(2):
                            f2mm(kp2, st, h, wt)
                else:
                    for st in sts:
                        for h in range(2):
                            f2mm(kp2, st, h, wt, stop=True)
                        # epilogue streams per s-tile as its chains end
                        s2 = stage.tile([P, 2], FP32, tag="s01", name="s01",
                                        bufs=8)
                        if trivial_gb:
                            # pre-LN2 = z + (beta+b2) + f2/SCL2 (beta_b2
                            # pre-added above)
                            for h in range(2):
                                n0 = h * NF
                                nc.vector.scalar_tensor_tensor(
                                    out=res[st][:, n0:n0 + NF], in0=f2[(st, h)][:],
                                    scalar=1.0 / SCL2, in1=res[st][:, n0:n0 + NF],
                                    op0=AluOpType.mult, op1=AluOpType.add,
                                    accum_out=s2[:, h:h + 1],
                                )
                        else:
                            t2 = stage.tile([P, D], FP32, tag="stgf2", name="stgf2")
                            nc.vector.tensor_mul(t2[:], res[st][:], bcast["gamma"][:])
                            for h in range(2):
                                n0 = h * NF
                                nc.vector.scalar_tensor_tensor(
                                    out=t2[:, n0:n0 + NF], in0=f2[(st, h)][:],
                                    scalar=1.0 / SCL2, in1=t2[:, n0:n0 + NF],
                                    op0=AluOpType.mult, op1=AluOpType.add,
                                )
                            nc.vector.tensor_add(res[st][:], t2[:],
                                                 bcast["beta_b2"][:])
                            nc.vector.reduce_sum(s2[:, 0:1], res[st][:, 0:NF])
                            nc.vector.reduce_sum(s2[:, 1:2], res[st][:, NF:D])
                        # LN2 + store; apply/store per d-half so the final
                        # out DMA starts half a tile earlier
                        nm, sd = ln_moments(st, s2)
                        ot = stage.tile([P, D], FP32, tag="stgf", name="stgf",
                                        bufs=3)
                        for h in range(2):
                            n0 = h * NF
                            nc.vector.tensor_scalar(
                                ot[:, n0:n0 + NF], res[st][:, n0:n0 + NF], nm, sd,
                                op0=AluOpType.add, op1=AluOpType.mult,
                            )
                            if not trivial_gb:
                                nc.vector.tensor_mul(ot[:, n0:n0 + NF],
                                                     ot[:, n0:n0 + NF],
                                                     bcast["gamma"][:, n0:n0 + NF])
                                nc.vector.tensor_add(ot[:, n0:n0 + NF],
                                                     ot[:, n0:n0 + NF],
                                                     bcast["beta"][:, n0:n0 + NF])
                            nc.gpsimd.dma_start(
                                out=ext["out_ext"][st * P:(st + 1) * P,
                                                   n0:n0 + NF],
                                in_=ot[:, n0:n0 + NF],
                            )


def build_nc(trivial_gb):
    nc = bacc.Bacc(target_bir_lowering=False, num_devices=N_CORES)

    ext = {
        "x_ext": nc.declare_dram_parameter("x", [S_LOC, D], FP32, isOutput=False),
        "wq8_ext": nc.declare_dram_parameter("wq8", [KP * P, 2 * D], FP8, isOutput=False),
        "wk8_ext": nc.declare_dram_parameter("wk8", [KP * P, 2 * D], FP8, isOutput=False),
        "wv8_ext": nc.declare_dram_parameter("wv8", [KP * P, 2 * D], FP8, isOutput=False),
        "w18_ext": nc.declare_dram_parameter("w18", [HG * KP * P, 2 * D], FP8, isOutput=False),
        "w28_ext": nc.declare_dram_parameter("w28", [(HT // 2) * P, 2 * D], FP8, isOutput=False),
        "bqp_ext": nc.declare_dram_parameter("bqp", [P, DT], FP32, isOutput=False),
        "bkp_ext": nc.declare_dram_parameter("bkp", [P, DT], FP32, isOutput=False),
        "b1p_ext": nc.declare_dram_parameter("b1p", [P, HT], FP32, isOutput=False),
        "bv_ext": nc.declare_dram_parameter("bv", [1, D], FP32, isOutput=False),
        "beta_b2_ext": nc.declare_dram_parameter("beta_b2", [1, D], FP32, isOutput=False),
        "gamma_ext": nc.declare_dram_parameter("gamma", [1, D], FP32, isOutput=False),
        "beta_ext": nc.declare_dram_parameter("beta", [1, D], FP32, isOutput=False),
        "out_ext": nc.declare_dram_parameter("out", [S_LOC, D], FP32, isOutput=True),
    }

    with tile.TileContext(nc) as tc:
        with (
            tc.tile_pool(name="dram", bufs=1, space="DRAM") as dram,
            tc.tile_pool(name="const", bufs=1) as const,
            tc.tile_pool(name="persist", bufs=1) as persist,
            tc.tile_pool(name="stage", bufs=2) as stage,
            tc.tile_pool(name="stream", bufs=2) as stream,
        ):
            ext.update(tc=tc, dram=dram, const=const, persist=persist,
                       stage=stage, stream=stream)
            build_graph(nc, tc, ext, trivial_gb)
    nc.compile()
    return nc


_NC_CACHE = {}


def _get_nc(trivial_gb):
    if trivial_gb not in _NC_CACHE:
        _NC_CACHE[trivial_gb] = build_nc(trivial_gb)
    return _NC_CACHE[trivial_gb]


F8NP = ml_dtypes.float8_e4m3


def _pair_rows(w):
    # [K, N] -> pair layout: rows kp*128+p, cols i*N+c = w[(2kp+i)*128+p, c]
    k, n = w.shape
    kp = k // (2 * P)
    w4 = w.reshape(kp, 2, P, n).transpose(0, 2, 1, 3).reshape(kp * P, 2 * n)
    return np.ascontiguousarray(w4)


def _col_pack(v, n):
    # [n*128] -> [128, n] with out[p, m] = v[m*128 + p]
    return np.ascontiguousarray(v.reshape(n, P).T)


def _make_in_maps(inputs):
    x = np.asarray(inputs["input_embedding"], dtype=np.float32)
    assert x.shape == (B, S, D), x.shape

    gamma = np.asarray(inputs["gamma"], np.float32).reshape(D)
    beta = np.asarray(inputs["beta"], np.float32).reshape(D)
    trivial_gb = bool(np.all(gamma == 1.0) and np.all(beta == 0.0))
    W1 = np.asarray(inputs["W1"], np.float32)
    b1 = np.asarray(inputs["b1"], np.float32).reshape(H)
    # fold LN1's gamma/beta into W1/b1 (FFN1 consumes the normalized z)
    W1f = gamma[:, None] * W1
    b1f = b1 + beta @ W1
    # W1 group-major pair layout: rows (g*KP+kp)*128+p, cols i*D+c
    w1g = (SCL * W1f).reshape(KP, 2, P, HG, D).transpose(3, 0, 2, 1, 4)
    w18 = np.ascontiguousarray(w1g.reshape(HG * KP * P, 2 * D)).astype(F8NP)

    shared = {
        "wq8": _pair_rows(SCL * np.asarray(inputs["Wq"], np.float32)).astype(F8NP),
        "wk8": _pair_rows(SCL * np.asarray(inputs["Wk"], np.float32)).astype(F8NP),
        "wv8": _pair_rows(SCL * np.asarray(inputs["Wv"], np.float32)).astype(F8NP),
        "w18": w18,
        "w28": _pair_rows(SCL2 * np.asarray(inputs["W2"], np.float32)).astype(F8NP),
        "bqp": _col_pack(SCL * np.asarray(inputs["bq"], np.float32).reshape(D), DT),
        "bkp": _col_pack(SCL * np.asarray(inputs["bk"], np.float32).reshape(D), DT),
        "b1p": _col_pack(b1f, HT),
        "bv": SCL * np.asarray(inputs["bv"], np.float32).reshape(1, D),
        "beta_b2": (beta + np.asarray(inputs["b2"], np.float32).reshape(D)).reshape(1, D),
        "gamma": gamma.reshape(1, D),
        "beta": beta.reshape(1, D),
    }

    in_maps = []
    for c in range(N_CORES):
        b = c // G
        r = c % G
        m = dict(shared)
        m["x"] = np.ascontiguousarray(x[b, r * S_LOC:(r + 1) * S_LOC, :])
        in_maps.append(m)
    return in_maps, trivial_gb


def kernel(**inputs: np.ndarray) -> np.ndarray:
    from concourse.bass_utils import run_bass_kernel_spmd

    in_maps, trivial_gb = _make_in_maps(inputs)
    nc = _get_nc(trivial_gb)
    res = run_bass_kernel_spmd(nc, in_maps, core_ids=list(range(N_CORES)))

    out = np.empty((B, S, D), dtype=np.float32)
    for c in range(N_CORES):
        b = c // G
        r = c % G
        out[b, r * S_LOC:(r + 1) * S_LOC, :] = res.results[c]["out"]
    return out


# revision 50
# speedup vs baseline: 1.1543x; 1.0060x over previous
"""Distributed Trainium2 kernel for a transformer attention block (B=2, S=4096,
D=1024, H=4096, fp32 I/O).

Reference computation (note the Q<-k, K<-q, V<-v argument quirk):
    k = x @ Wk + bk ; q = x @ Wq + bq ; v = x @ Wv + bv
    scores[s,t] = k[s]·q[t] / sqrt(D); attn = softmax_t(scores) @ v
    x1 = LN(x + attn); h = gelu(x1 @ W1 + b1); out = LN(x1 + h @ W2 + b2)

Sharding: 8 cores -> 2 groups of 4 (one group per batch element); each core
owns 1024 sequence rows. Design notes:
  - all five GEMMs run fp8 (e4m3) DoubleRow matmuls. Weights are pre-cast/
    pre-tiled on the host into the DoubleRow pair layout. Host pre-scales
    Wq/Wk/Wv/W1 by 32 and W2 by 64; inverse scales fold into activation
    scale constants.
  - a leading dummy AllGather absorbs program-launch skew so the real q/v
    AllGathers handshake fast. All gather-input stores and gathered-output
    loads are issued from the GpSimd queue so the Sync queue (x/weight
    loads) never blocks behind a collective wait, and the collective
    triggers sequence naturally behind their input stores.
  - LayerNorms run as moment sums: the PSUM->SBUF evictions carry
    accum_out partials (sum x), a scalar-engine Square pass accumulates
    sum x^2, and the tiny per-partition algebra runs on vector. LN1 is
    interleaved into pass B's second half so its transposes slot into the
    PE stream while vector/scalar overlap the attention matmuls.
  - FFN1 streams W1 once (group-outer, s-half-inner). FFN2 runs 2 passes
    of 4 s-tiles (all 8 PSUM banks); W2 k-tiles 13..15 stay resident so
    each pass's chains START on them and the per-s-tile chain ends stagger
    on the last streamed tile, letting epilogues drain while the next
    pass's resident matmuls keep the PE warm.
"""

import sys

if "/opt/trn_rl_repo" not in sys.path:
    sys.path.insert(0, "/opt/trn_rl_repo")

import numpy as np
import ml_dtypes

import concourse.bacc as bacc
import concourse.mybir as mybir
import concourse.tile as tile
from concourse.alu_op_type import AluOpType
from concourse.masks import make_identity


AF = mybir.ActivationFunctionType
FP32 = mybir.dt.float32
BF16 = mybir.dt.bfloat16
FP8 = mybir.dt.float8e4
DR = mybir.MatmulPerfMode.DoubleRow

B, S, D, H = 2, 4096, 1024, 4096
N_CORES = 8
G = 4                 # cores per group (one group per batch element)
S_LOC = S // G        # sequence rows per core
P = 128               # SBUF partitions
NF = 512              # matmul moving free-dim (one fp32 PSUM bank)
DT = D // P           # 8 d-tiles
KP = DT // 2          # 4 k-subtile pairs over D
ST = S_LOC // P       # 8 s-tiles per core
TJ = S // P           # 32 global t-subtiles
HT = H // P           # 32 h-tiles
HG = 4                # FFN1 weight-streaming groups
HPG = HT // HG        # 8 h-tiles per group
W2R = 3               # resident W2 tail k-pair tiles (13, 14, 15)
W2S = HT // 2 - W2R   # streamed W2 k-pair tiles per pass (0..12)
EPS = 1e-5
SCL = 32.0            # host pre-scale on Wq/Wk/Wv/W1
SCL2 = 64.0           # host pre-scale on W2
SM_SCALE = 1.0 / float(np.sqrt(np.float32(D)))
EXP_SCALE = SM_SCALE / (SCL * SCL)

GROUPS = [[0, 1, 2, 3], [4, 5, 6, 7]]


def build_graph(nc, tc, ext, trivial_gb):
    stream = ext["stream"]
    persist = ext["persist"]
    stage = ext["stage"]
    const = ext["const"]
    dram = ext["dram"]
    tcx = ext["tc"]

    # ---- constants ----
    ident_bf = const.tile([P, P], BF16, tag="ident_bf", name="ident_bf")
    make_identity(nc, ident_bf[:])
    ident_f = const.tile([P, P], FP32, tag="ident_f", name="ident_f")
    make_identity(nc, ident_f[:])
    ones_dr = const.tile([P, 2, 16], FP8, tag="ones_dr", name="ones_dr")
    nc.vector.memset(ones_dr[:, :, :], 1.0)
    ones_f32 = const.tile([1, P], FP32, tag="ones_f32", name="ones_f32")
    nc.vector.memset(ones_f32[:], 1.0)
    eps_t = const.tile([P, 1], FP32, tag="eps", name="eps")
    nc.vector.memset(eps_t[:], EPS)

    # biases arrive host-packed: [P, 8] bq | [P, 8] bk | [P, 32] b1.
    # (Their DMAs are issued after the first x loads so the cold-start x
    # chunk leads the Sync queue.)
    pvecs = const.tile([P, 48], FP32, tag="pvecs", name="pvecs")
    bq_sb = [pvecs[:, m:m + 1] for m in range(DT)]
    bk_sb = [pvecs[:, DT + m:DT + m + 1] for m in range(DT)]
    b1_sb = [pvecs[:, 2 * DT + m:2 * DT + m + 1] for m in range(HT)]

    # free-dim [1, D] rows at 32-aligned partitions (matmul-legal bases);
    # row 96 is scratch for the softmax recip row (never a matmul operand)
    smalls = const.tile([P, D], FP32, tag="smalls", name="smalls")
    SROW = {"bv": 0, "gamma": 32, "beta": 64}
    smalls2 = const.tile([1, D], FP32, tag="smalls2", name="smalls2")

    def load_small_consts():
        nc.sync.dma_start(out=pvecs[:, 0:DT], in_=ext["bqp_ext"][:, :])
        nc.sync.dma_start(out=pvecs[:, DT:2 * DT], in_=ext["bkp_ext"][:, :])
        nc.sync.dma_start(out=pvecs[:, 2 * DT:2 * DT + HT], in_=ext["b1p_ext"][:, :])
        for nm, r in SROW.items():
            nc.sync.dma_start(out=smalls[r:r + 1, :], in_=ext[nm + "_ext"][0:1, :])
        nc.sync.dma_start(out=smalls2[0:1, :], in_=ext["beta_b2_ext"][0:1, :])

    res = [persist.tile([P, D], FP32, tag=f"res{m}", name=f"res{m}") for m in range(ST)]
    xT_f8 = persist.tile([P, DT, S_LOC], FP8, tag="xT", name="xT")
    qT_f8 = persist.tile([P, DT, S_LOC], FP8, tag="qT", name="qT")
    kT_f8 = persist.tile([P, DT, S_LOC], FP8, tag="kT", name="kT")
    v_half = [persist.tile([P, TJ, NF], FP8, tag=f"vf{h}", name=f"vf{h}")
              for h in range(2)]
    P_f8 = persist.tile([P, TJ, S_LOC], FP8, tag="pf", name="pf")
    # resident W2 tail k-pair tiles (loaded during phase A, used by both
    # FFN2 passes), and FFN1's last weight group (its streamed slot would
    # WAR-stall against group 0's matmuls right at the FFN1->FFN2 seam)
    w2r = [persist.tile([P, 2, D], FP8, tag=f"w2r{i}", name=f"w2r{i}")
           for i in range(W2R)]
    w1g3r = [persist.tile([P, 2, D], FP8, tag=f"w1g3r{i}", name=f"w1g3r{i}")
             for i in range(KP)]

    # gather buffers are partition-major ([p, tile, col] flattened) so the
    # post-gather loads are single DMAs with 4KB-contiguous runs
    ag_q_in = [dram.tile([P, DT * NF], FP8, name=f"agqi{h}") for h in range(2)]
    ag_q_out = [dram.tile([G * P, DT * NF], FP8, name=f"agqo{h}") for h in range(2)]
    ag_v_in = [dram.tile([P, ST * NF], FP8, name=f"agvi{h}") for h in range(2)]
    ag_v_out = [dram.tile([G * P, ST * NF], FP8, name=f"agvo{h}") for h in range(2)]

    bcast = {}
    recipT = const.tile([P, ST], FP32, tag="recipT", name="recipT")

    def load_w8(ext_t, base_row, eng=None):
        eng = eng or nc.sync
        tiles = []
        for kp in range(KP):
            wt = stream.tile([P, 2, D], FP8, tag=f"w{kp}", name=f"w{kp}")
            r0 = base_row + kp * P
            eng.dma_start(out=wt[:, :, :], in_=ext_t[r0:r0 + P, :])
            tiles.append(wt)
        return tiles

    # LayerNorm via moments: sums arrive via accum_out on the evictions
    # (s01), sum-of-squares via a scalar-engine Square pass. Returns
    # (negmu*sd, sd) for the fused apply  (x + nm/sd)*sd = (x-mu)*sd.
    def ln_moments(st, s01):
        lt = stage.tile([P, 8], FP32, tag="lnt", name="lnt", bufs=3)
        sq = stage.tile([P, D], FP32, tag="sqt", name="sqt", bufs=1)
        sumsq = lt[:, 0:1]
        nc.scalar.activation(sq[:], res[st][:], AF.Square, accum_out=sumsq)
        ssum = lt[:, 1:2]
        nc.vector.tensor_add(ssum, s01[:, 0:1], s01[:, 1:2])
        negmu = lt[:, 2:3]
        nc.vector.tensor_scalar_mul(negmu, ssum, -1.0 / D)
        var = lt[:, 3:4]
        mu2 = lt[:, 4:5]
        nc.vector.tensor_mul(mu2, negmu, negmu)
        nc.vector.scalar_tensor_tensor(
            out=var, in0=sumsq, scalar=1.0 / D, in1=mu2,
            op0=AluOpType.mult, op1=AluOpType.subtract,
        )
        sd = lt[:, 5:6]
        nc.scalar.activation(sd, var, AF.Sqrt, bias=eps_t[:])
        nc.vector.reciprocal(sd, sd)
        nm = lt[:, 6:7]
        nc.vector.tensor_mul(nm, negmu, sd)
        return nm, sd

    # evict_eng picks which engine pays for the PSUM->SBUF fp8 eviction
    # (vector during the x loads, scalar inside pass B where vector is the
    # bottleneck)
    def transpose_to(mmp, src_bf, dst_f8, s0, evict_eng=None):
        evict = evict_eng or nc.vector
        tp = mmp.tile([P, DT * P], BF16, tag="trp", name="trp", bufs=2)
        for dj in range(DT):
            nc.tensor.transpose(
                tp[:, dj * P:(dj + 1) * P], src_bf[:, dj * P:(dj + 1) * P],
                ident_bf[:],
            )
        copy_fn = evict.copy if evict is nc.scalar else evict.tensor_copy
        copy_fn(
            out=dst_f8[:, :, s0:s0 + P],
            in_=tp[:].rearrange("p (d s) -> p d s", d=DT),
        )

    # ================= phase A: QKV, attention, LN1, FFN1 =================
    with tcx.tile_pool(name="psA", bufs=1, space="PSUM") as mmp:
        # PE warm-up: ~40 dependency-free matmuls while the first x DMA is
        # in flight. The HAM clock gate needs ~3.4us of sustained PE
        # activity before it lifts the 1.2GHz cold throttle — burning that
        # window on throwaway work makes the real QKV matmuls run at 2.4GHz.
        for _ in range(64):
            wp = mmp.tile([P, NF], FP32, tag="mm", name="mm", bufs=4)
            nc.tensor.matmul(wp[:, 0:P], ident_bf[:], ident_bf[:])
        # ---- x -> xT fp8: first s-half, then q-half0 can go ----
        # loads alternate between the Sync and Scalar queues so the 4MB of
        # x doesn't trickle through a single DMA ring at cold start
        x_eng = [nc.sync, nc.scalar, nc.sync, nc.scalar]

        def load_x_half(h):
            for si in range(h * 4, h * 4 + 4):
                xn = stage.tile([P, D], FP32, tag="stgf", name="stgf", bufs=3)
                xb = stage.tile([P, D], BF16, tag="stgb", name="stgb")
                eng = x_eng[si % 4]
                if si == 0:
                    # split the first tile so the cold-start PE work begins
                    # after ~256KB instead of ~512KB of DMA
                    for c in range(2):
                        eng.dma_start(
                            out=xn[:, c * NF:(c + 1) * NF],
                            in_=ext["x_ext"][si * P:(si + 1) * P, c * NF:(c + 1) * NF],
                        )
                        nc.vector.tensor_copy(
                            out=xb[:, c * NF:(c + 1) * NF],
                            in_=xn[:, c * NF:(c + 1) * NF],
                        )
                else:
                    eng.dma_start(out=xn[:], in_=ext["x_ext"][si * P:(si + 1) * P, :])
                    nc.vector.tensor_copy(out=xb[:], in_=xn[:])
                transpose_to(mmp, xb, xT_f8, si * P)

        def q_half(h):
            n0 = h * NF
            for m in range(DT):
                pt = mmp.tile([P, NF], FP32, tag="mm", name="mm", bufs=4)
                for kp in range(KP):
                    nc.tensor.matmul(
                        pt[:], wq[kp][:, :, m * P:(m + 1) * P],
                        xT_f8[:, 2 * kp:2 * kp + 2, n0:n0 + NF],
                        start=(kp == 0), stop=(kp == KP - 1), perf_mode=DR,
                    )
                nc.scalar.activation(qT_f8[:, m, n0:n0 + NF], pt[:], AF.Identity,
                                     bias=bq_sb[m])
                nc.gpsimd.dma_start(
                    out=ag_q_in[h][:, m * NF:(m + 1) * NF],
                    in_=qT_f8[:, m, n0:n0 + NF],
                )
            nc.gpsimd.collective_compute(
                "AllGather", AluOpType.bypass, replica_groups=GROUPS,
                ins=[ag_q_in[h][:].opt()], outs=[ag_q_out[h][:].opt()],
            )

        load_x_half(0)
        wq = load_w8(ext["wq8_ext"], 0, eng=nc.scalar)
        load_small_consts()
        q_half(0)
        load_x_half(1)
        q_half(1)

        # ---- v = x @ (32 Wv) + 32 bv (natural, fp8); AllGather ----
        wv = load_w8(ext["wv8_ext"], 0)
        bv_b = const.tile([P, D], FP32, tag="bc_bv", name="bc_bv")
        for n0 in range(0, D, NF):
            pt = mmp.tile([P, NF], FP32, tag="mm", name="mm", bufs=4)
            nc.tensor.matmul(pt[:], ones_f32[0:1, :], smalls[0:1, n0:n0 + NF])
            nc.scalar.copy(out=bv_b[:, n0:n0 + NF], in_=pt[:])
        for mt in range(ST):
            v8 = stage.tile([P, D], FP8, tag="v8", name="v8")
            for n0 in range(0, D, NF):
                pt = mmp.tile([P, NF], FP32, tag="mm", name="mm", bufs=4)
                for kp in range(KP):
                    nc.tensor.matmul(
                        pt[:], xT_f8[:, 2 * kp:2 * kp + 2, mt * P:(mt + 1) * P],
                        wv[kp][:, :, n0:n0 + NF],
                        start=(kp == 0), stop=(kp == KP - 1), perf_mode=DR,
                    )
                nc.vector.tensor_add(
                    v8[:, n0:n0 + NF], pt[:], bv_b[:, n0:n0 + NF]
                )
            for hh in range(2):
                nc.gpsimd.dma_start(
                    out=ag_v_in[hh][:, mt * NF:(mt + 1) * NF],
                    in_=v8[:, hh * NF:(hh + 1) * NF],
                )
        # v gathered in two d-halves so pass B's first half can start while
        # the second half is still on the wire (the CC stream is serial)
        for hh in range(2):
            nc.gpsimd.collective_compute(
                "AllGather", AluOpType.bypass, replica_groups=GROUPS,
                ins=[ag_v_in[hh][:].opt()], outs=[ag_v_out[hh][:].opt()],
            )

        # ---- kT = (32 Wk).T @ x + 32 bk (fp8, local) ----
        wk = load_w8(ext["wk8_ext"], 0)
        # resident W2 tail tiles + FFN1's last weight group: load now on the
        # Scalar queue (no sem waits there) so the Sync queue's qch stream
        # leads right at the q0-mesh end (consumed in FFN1/FFN2)
        for i in range(W2R):
            kp2 = W2S + i
            nc.scalar.dma_start(
                out=w2r[i][:, :, :], in_=ext["w28_ext"][kp2 * P:(kp2 + 1) * P, :]
            )
        for kp in range(KP):
            r0 = ((HG - 1) * KP + kp) * P
            nc.scalar.dma_start(
                out=w1g3r[kp][:, :, :], in_=ext["w18_ext"][r0:r0 + P, :]
            )
        for m in range(DT):
            for n0 in range(0, S_LOC, NF):
                pt = mmp.tile([P, NF], FP32, tag="mm", name="mm", bufs=4)
                for kp in range(KP):
                    nc.tensor.matmul(
                        pt[:], wk[kp][:, :, m * P:(m + 1) * P],
                        xT_f8[:, 2 * kp:2 * kp + 2, n0:n0 + NF],
                        start=(kp == 0), stop=(kp == KP - 1), perf_mode=DR,
                    )
                nc.scalar.activation(kT_f8[:, m, n0:n0 + NF], pt[:], AF.Identity,
                                     bias=bk_sb[m])

        # [P, D] broadcasts, off the critical path (fills AG wait)
        bc_rows = [("gamma", smalls[32:33, :]),
                   ("beta", smalls[64:65, :]),
                   ("beta_b2", smalls2[0:1, :])]
        if trivial_gb:
            bc_rows = [bc_rows[2]]  # only beta+b2 needed
        for nm, srow in bc_rows:
            bt = const.tile([P, D], FP32, tag=f"bc_{nm}", name=f"bc_{nm}")
            for n0 in range(0, D, NF):
                pt = mmp.tile([P, NF], FP32, tag="mm", name="mm", bufs=4)
                nc.tensor.matmul(pt[:], ones_f32[0:1, :], srow[:, n0:n0 + NF])
                nc.scalar.copy(out=bt[:, n0:n0 + NF], in_=pt[:])
            bcast[nm] = bt

        # ---- pass A: P[t, s] = exp(k·q/sqrt(D)); DR rowsums 1 chunk back ----
        rs_ps = [mmp.tile([1, NF], FP32, tag=f"rs{h}", name=f"rs{h}", bufs=1)
                 for h in range(2)]
        chunks = [(ht, r) for ht in range(2) for r in range(G)]

        def emit_rowsum(ci):
            ht, r = chunks[ci]
            jp0 = (r * ST + ht * 4) // 2
            for h in range(2):
                n0 = h * NF
                for jj in range(2):
                    a = 2 * ci + jj
                    nc.tensor.matmul(
                        rs_ps[h][:], ones_dr[:, :, 0:1],
                        P_f8[:, 2 * (jp0 + jj):2 * (jp0 + jj) + 2, n0:n0 + NF],
                        start=(a == 0), stop=(a == 2 * len(chunks) - 1),
                        perf_mode=DR,
                    )

        qtiles = {}

        def issue_qch(ci):
            ht, r = chunks[ci]
            qch = stream.tile([P, DT, NF], FP8, tag="q", name="q", bufs=3)
            nc.sync.dma_start(
                out=qch[:, :, :], in_=ag_q_out[ht][r * P:(r + 1) * P, :]
            )
            qtiles[ci] = qch

        issue_qch(0)
        issue_qch(1)
        for ci, (ht, r) in enumerate(chunks):
            if ci + 2 < len(chunks):
                issue_qch(ci + 2)
            qch = qtiles.pop(ci)
            for tti in range(4):
                j = r * ST + ht * 4 + tti
                for n0 in range(0, S_LOC, NF):
                    ps = mmp.tile([P, NF], FP32, tag="mm", name="mm", bufs=4)
                    for kp in range(KP):
                        nc.tensor.matmul(
                            ps[:], qch[:, 2 * kp:2 * kp + 2, tti * P:(tti + 1) * P],
                            kT_f8[:, 2 * kp:2 * kp + 2, n0:n0 + NF],
                            start=(kp == 0), stop=(kp == KP - 1), perf_mode=DR,
                        )
                    nc.scalar.activation(
                        P_f8[:, j, n0:n0 + NF], ps[:], AF.Exp, scale=EXP_SCALE
                    )
            if ci > 0:
                emit_rowsum(ci - 1)
        emit_rowsum(len(chunks) - 1)

        # raw rowsums -> smalls row 96 (scalar copies: a [1,512] vector op
        # is single-partition and takes ~3.3us); the reciprocal runs after
        # the transpose in [P, 8] form where it's ~100x faster. The tiny
        # transpose is emitted inside pass B (after st0's matmuls) so the
        # PE queue doesn't stall on it before the attention matmuls.
        rs_row = smalls[96:97, :]
        for h in range(2):
            nc.scalar.copy(out=rs_row[0:1, h * NF:(h + 1) * NF], in_=rs_ps[h][:])
        rs8 = const.tile([ST, P], FP32, tag="rs8", name="rs8")
        nc.scalar.dma_start(out=rs8[:, :], in_=rs_row[0:1, :])

        # ---- pass B: attn natural [s, d] + residual -> res (fp32) ----
        # d-half outer: half 0 computes while v's half-1 gather is in flight.
        # gathered-v loads ride the GpSimd queue (idle after the triggers) so
        # the Sync queue's qch stream can't delay them.
        for hh in range(2):
            nc.gpsimd.dma_start(
                out=v_half[hh][:, :, :].rearrange("p (r m) c -> p r (m c)", r=G),
                in_=ag_v_out[hh][:, :].rearrange("(r p) c -> p r c", p=P),
            )

        # LN1 interleaved into the h=1 evictions: stats/apply/cast overlap
        # the next s-tile's attention matmuls; transposes slot into the PE
        # stream.
        x1T_f8 = persist.tile([P, DT, S_LOC], FP8, tag="xT", name="xT")
        h_sh = [persist.tile([P, TJ, NF], FP8, tag=f"vf{h}", name=f"vf{h}")
                for h in range(2)]

        s01 = {}
        for h in range(2):
            n0 = h * NF
            for st in range(ST):
                xre = stage.tile([P, NF], FP32, tag="xre", name="xre")
                nc.scalar.dma_start(
                    out=xre[:], in_=ext["x_ext"][st * P:(st + 1) * P, n0:n0 + NF]
                )
                ps = mmp.tile([P, NF], FP32, tag="mm", name="mm", bufs=4)
                for jp in range(TJ // 2):
                    nc.tensor.matmul(
                        ps[:], P_f8[:, 2 * jp:2 * jp + 2, st * P:(st + 1) * P],
                        v_half[h][:, 2 * jp:2 * jp + 2, :],
                        start=(jp == 0), stop=(jp == TJ // 2 - 1), perf_mode=DR,
                    )
                if h == 0 and st == 0:
                    rt_ps = mmp.tile([P, NF], FP32, tag="mm", name="mm", bufs=4)
                    nc.tensor.transpose(rt_ps[:, 0:ST], rs8[:, :],
                                        ident_f[0:ST, 0:ST])
                    nc.scalar.activation(recipT[:], rt_ps[:, 0:ST], AF.Identity,
                                         scale=SCL)
                    nc.vector.reciprocal(recipT[:], recipT[:])
                if h == 0:
                    sx = stage.tile([P, 2], FP32, tag="s01", name="s01", bufs=8)
                    s01[st] = sx
                    acc = sx[:, 0:1]
                else:
                    acc = s01[st][:, 1:2]
                nc.vector.scalar_tensor_tensor(
                    out=res[st][:, n0:n0 + NF], in0=ps[:], scalar=recipT[:, st:st + 1],
                    in1=xre[:], op0=AluOpType.mult, op1=AluOpType.add,
                    accum_out=acc,
                )
                if h == 1:
                    # LN1: res[st] <- z (normalized); then cast+transpose
                    nm, sd = ln_moments(st, s01[st])
                    nc.vector.tensor_scalar(
                        res[st][:], res[st][:], nm, sd,
                        op0=AluOpType.add, op1=AluOpType.mult,
                    )
                    xb = stage.tile([P, D], BF16, tag="stgb", name="stgb")
                    nc.vector.tensor_copy(out=xb[:], in_=res[st][:])
                    transpose_to(mmp, xb, x1T_f8, st * P, evict_eng=nc.scalar)

        # ---- FFN1: stream W1 once (group-outer, s-half-inner) ----
        for g in range(HG):
            w1g = (w1g3r if g == HG - 1
                   else load_w8(ext["w18_ext"], g * KP * P))
            for sh in range(2):
                n0 = sh * NF
                for mh_i in range(HPG):
                    mh = g * HPG + mh_i
                    pt = mmp.tile([P, NF], FP32, tag="mm", name="mm", bufs=4)
                    for kp in range(KP):
                        nc.tensor.matmul(
                            pt[:], w1g[kp][:, :, mh_i * P:(mh_i + 1) * P],
                            x1T_f8[:, 2 * kp:2 * kp + 2, n0:n0 + NF],
                            start=(kp == 0), stop=(kp == KP - 1), perf_mode=DR,
                        )
                    nc.scalar.activation(
                        h_sh[sh][:, mh, :], pt[:], AF.Gelu,
                        bias=b1_sb[mh], scale=1.0 / SCL,
                    )

    # ================= phase B: FFN2 (fp8 DR) + LN2 + out =================
    # 3 passes of (3, 3, 2) s-tiles. Chains START on the resident W2 tail
    # tiles (which also overlap the previous pass's epilogues), then
    # consume streamed k 0..12; per-s-tile chain ends stagger on k=12 so
    # the epilogues pipeline, and the 2-tile last pass keeps the final
    # epilogue tail short.
    with tcx.tile_pool(name="psB", bufs=1, space="PSUM") as f2p:
        for sp, sts in enumerate([(0, 1, 2), (3, 4, 5), (6, 7)]):
            f2 = {(st, h): f2p.tile([P, NF], FP32, tag=f"f{i}_{h}",
                                    name=f"f{i}_{h}")
                  for i, st in enumerate(sts) for h in range(2)}

            def f2mm(kp2, st, h, w2t, start=False, stop=False):
                nc.tensor.matmul(
                    f2[(st, h)][:],
                    h_sh[st // 4][:, 2 * kp2:2 * kp2 + 2,
                                  (st % 4) * P:(st % 4 + 1) * P],
                    w2t[:, :, h * NF:(h + 1) * NF],
                    start=start, stop=stop, perf_mode=DR,
                )

            # resident tail first: starts the accumulation groups
            for st in sts:
                for h in range(2):
                    for i in range(W2R):
                        f2mm(W2S + i, st, h, w2r[i], start=(i == 0))

            # streamed k-pairs 0..W2S-1; stagger chain ends on the last one
            for kp2 in range(W2S):
                wt = stream.tile([P, 2, D], FP8, tag=f"w{kp2 % KP}",
                                 name=f"w{kp2 % KP}")
                nc.sync.dma_start(
                    out=wt[:, :, :],
                    in_=ext["w28_ext"][kp2 * P:(kp2 + 1) * P, :],
                )
                if kp2 == 0 and trivial_gb:
                    # pre-add beta+b2 into res here (vector is idle during
                    # the matmul stream) so the per-tile tail epilogue shrinks
                    for st in sts:
                        nc.vector.tensor_add(res[st][:], res[st][:],
                                             bcast["beta_b2"][:])
                if kp2 < W2S - 1:
                    for st in sts:
                        for h in range(2):
                            f2mm(kp2, st, h, wt)
                else:
                    for st in sts:
                        for h in range(2):
                            f2mm(kp2, st, h, wt, stop=True)
                        # epilogue streams per s-tile as its chains end
                        s2 = stage.tile([P, 2], FP32, tag="s01", name="s01",
                                        bufs=8)
                        if trivial_gb:
                            # pre-LN2 = z + (beta+b2) + f2/SCL2 (beta_b2
                            # pre-added above)
                            for h in range(2):
                                n0 = h * NF
                                nc.vector.scalar_tensor_tensor(
                                    out=res[st][:, n0:n0 + NF], in0=f2[(st, h)][:],
                                    scalar=1.0 / SCL2, in1=res[st][:, n0:n0 + NF],
                                    op0=AluOpType.mult, op1=AluOpType.add,
                                    accum_out=s2[:, h:h + 1],
                                )
                        else:
                            t2 = stage.tile([P, D], FP32, tag="stgf2", name="stgf2")
                            nc.vector.tensor_mul(t2[:], res[st][:], bcast["gamma"][:])
                            for h in range(2):
                                n0 = h * NF
                                nc.vector.scalar_tensor_tensor(
                                    out=t2[:, n0:n0 + NF], in0=f2[(st, h)][:],
                                    scalar=1.0 / SCL2, in1=t2[:, n0:n0 + NF],
                                    op0=AluOpType.mult, op1=AluOpType.add,
                                )
                            nc.vector.tensor_add(res[st][:], t2[:],
                                                 bcast["beta_b2"][:])
                            nc.vector.reduce_sum(s2[:, 0:1], res[st][:, 0:NF])
                            nc.vector.reduce_sum(s2[:, 1:2], res[st][:, NF:D])
                        # LN2 + store; the apply runs on scalar (vector is
                        # the epilogue bottleneck), per d-half so the final
                        # out DMA starts half a tile earlier
                        nm, sd = ln_moments(st, s2)
                        ot = stage.tile([P, D], FP32, tag="stgf", name="stgf",
                                        bufs=3)
                        for h in range(2):
                            n0 = h * NF
                            nc.scalar.activation(
                                ot[:, n0:n0 + NF], res[st][:, n0:n0 + NF],
                                AF.Identity, bias=nm, scale=sd,
                            )
                            if not trivial_gb:
                                nc.vector.tensor_mul(ot[:, n0:n0 + NF],
                                                     ot[:, n0:n0 + NF],
                                                     bcast["gamma"][:, n0:n0 + NF])
                                nc.vector.tensor_add(ot[:, n0:n0 + NF],
                                                     ot[:, n0:n0 + NF],
                                                     bcast["beta"][:, n0:n0 + NF])
                            nc.gpsimd.dma_start(
                                out=ext["out_ext"][st * P:(st + 1) * P,
                                                   n0:n0 + NF],
                                in_=ot[:, n0:n0 + NF],
                            )


def build_nc(trivial_gb):
    nc = bacc.Bacc(target_bir_lowering=False, num_devices=N_CORES)

    ext = {
        "x_ext": nc.declare_dram_parameter("x", [S_LOC, D], FP32, isOutput=False),
        "wq8_ext": nc.declare_dram_parameter("wq8", [KP * P, 2 * D], FP8, isOutput=False),
        "wk8_ext": nc.declare_dram_parameter("wk8", [KP * P, 2 * D], FP8, isOutput=False),
        "wv8_ext": nc.declare_dram_parameter("wv8", [KP * P, 2 * D], FP8, isOutput=False),
        "w18_ext": nc.declare_dram_parameter("w18", [HG * KP * P, 2 * D], FP8, isOutput=False),
        "w28_ext": nc.declare_dram_parameter("w28", [(HT // 2) * P, 2 * D], FP8, isOutput=False),
        "bqp_ext": nc.declare_dram_parameter("bqp", [P, DT], FP32, isOutput=False),
        "bkp_ext": nc.declare_dram_parameter("bkp", [P, DT], FP32, isOutput=False),
        "b1p_ext": nc.declare_dram_parameter("b1p", [P, HT], FP32, isOutput=False),
        "bv_ext": nc.declare_dram_parameter("bv", [1, D], FP32, isOutput=False),
        "beta_b2_ext": nc.declare_dram_parameter("beta_b2", [1, D], FP32, isOutput=False),
        "gamma_ext": nc.declare_dram_parameter("gamma", [1, D], FP32, isOutput=False),
        "beta_ext": nc.declare_dram_parameter("beta", [1, D], FP32, isOutput=False),
        "out_ext": nc.declare_dram_parameter("out", [S_LOC, D], FP32, isOutput=True),
    }

    with tile.TileContext(nc) as tc:
        with (
            tc.tile_pool(name="dram", bufs=1, space="DRAM") as dram,
            tc.tile_pool(name="const", bufs=1) as const,
            tc.tile_pool(name="persist", bufs=1) as persist,
            tc.tile_pool(name="stage", bufs=2) as stage,
            tc.tile_pool(name="stream", bufs=2) as stream,
        ):
            ext.update(tc=tc, dram=dram, const=const, persist=persist,
                       stage=stage, stream=stream)
            build_graph(nc, tc, ext, trivial_gb)
    nc.compile()
    return nc


_NC_CACHE = {}


def _get_nc(trivial_gb):
    if trivial_gb not in _NC_CACHE:
        _NC_CACHE[trivial_gb] = build_nc(trivial_gb)
    return _NC_CACHE[trivial_gb]


F8NP = ml_dtypes.float8_e4m3


def _pair_rows(w):
    # [K, N] -> pair layout: rows kp*128+p, cols i*N+c = w[(2kp+i)*128+p, c]
    k, n = w.shape
    kp = k // (2 * P)
    w4 = w.reshape(kp, 2, P, n).transpose(0, 2, 1, 3).reshape(kp * P, 2 * n)
    return np.ascontiguousarray(w4)


def _col_pack(v, n):
    # [n*128] -> [128, n] with out[p, m] = v[m*128 + p]
    return np.ascontiguousarray(v.reshape(n, P).T)


def _make_in_maps(inputs):
    x = np.asarray(inputs["input_embedding"], dtype=np.float32)
    assert x.shape == (B, S, D), x.shape

    gamma = np.asarray(inputs["gamma"], np.float32).reshape(D)
    beta = np.asarray(inputs["beta"], np.float32).reshape(D)
    trivial_gb = bool(np.all(gamma == 1.0) and np.all(beta == 0.0))
    W1 = np.asarray(inputs["W1"], np.float32)
    b1 = np.asarray(inputs["b1"], np.float32).reshape(H)
    # fold LN1's gamma/beta into W1/b1 (FFN1 consumes the normalized z)
    W1f = gamma[:, None] * W1
    b1f = b1 + beta @ W1
    # W1 group-major pair layout: rows (g*KP+kp)*128+p, cols i*D+c
    w1g = (SCL * W1f).reshape(KP, 2, P, HG, D).transpose(3, 0, 2, 1, 4)
    w18 = np.ascontiguousarray(w1g.reshape(HG * KP * P, 2 * D)).astype(F8NP)

    shared = {
        "wq8": _pair_rows(SCL * np.asarray(inputs["Wq"], np.float32)).astype(F8NP),
        "wk8": _pair_rows(SCL * np.asarray(inputs["Wk"], np.float32)).astype(F8NP),
        "wv8": _pair_rows(SCL * np.asarray(inputs["Wv"], np.float32)).astype(F8NP),
        "w18": w18,
        "w28": _pair_rows(SCL2 * np.asarray(inputs["W2"], np.float32)).astype(F8NP),
        "bqp": _col_pack(SCL * np.asarray(inputs["bq"], np.float32).reshape(D), DT),
        "bkp": _col_pack(SCL * np.asarray(inputs["bk"], np.float32).reshape(D), DT),
        "b1p": _col_pack(b1f, HT),
        "bv": SCL * np.asarray(inputs["bv"], np.float32).reshape(1, D),
        "beta_b2": (beta + np.asarray(inputs["b2"], np.float32).reshape(D)).reshape(1, D),
        "gamma": gamma.reshape(1, D),
        "beta": beta.reshape(1, D),
    }

    in_maps = []
    for c in range(N_CORES):
        b = c // G
        r = c % G
        m = dict(shared)
        m["x"] = np.ascontiguousarray(x[b, r * S_LOC:(r + 1) * S_LOC, :])
        in_maps.append(m)
    return in_maps, trivial_gb


def kernel(**inputs: np.ndarray) -> np.ndarray:
    from concourse.bass_utils import run_bass_kernel_spmd

    in_maps, trivial_gb = _make_in_maps(inputs)
    nc = _get_nc(trivial_gb)
    res = run_bass_kernel_spmd(nc, in_maps, core_ids=list(range(N_CORES)))

    out = np.empty((B, S, D), dtype=np.float32)
    for c in range(N_CORES):
        b = c // G
        r = c % G
        out[b, r * S_LOC:(r + 1) * S_LOC, :] = res.results[c]["out"]
    return out


# revision 52
# speedup vs baseline: 1.1614x; 1.0062x over previous
"""Distributed Trainium2 kernel for a transformer attention block (B=2, S=4096,
D=1024, H=4096, fp32 I/O).

Reference computation (note the Q<-k, K<-q, V<-v argument quirk):
    k = x @ Wk + bk ; q = x @ Wq + bq ; v = x @ Wv + bv
    scores[s,t] = k[s]·q[t] / sqrt(D); attn = softmax_t(scores) @ v
    x1 = LN(x + attn); h = gelu(x1 @ W1 + b1); out = LN(x1 + h @ W2 + b2)

Sharding: 8 cores -> 2 groups of 4 (one group per batch element); each core
owns 1024 sequence rows. Design notes:
  - all five GEMMs run fp8 (e4m3) DoubleRow matmuls. Weights are pre-cast/
    pre-tiled on the host into the DoubleRow pair layout. Host pre-scales
    Wq/Wk/Wv/W1 by 32 and W2 by 64; inverse scales fold into activation
    scale constants.
  - all gather-input stores and gathered-output loads are issued from the
    GpSimd queue so the Sync queue (x/weight loads) never blocks behind a
    collective wait, and the collective triggers sequence naturally behind
    their input stores. x loads alternate Sync/Scalar queues, and a ~64
    matmul warm-up burst lifts the HAM 1.2GHz cold throttle while the
    first x DMA is in flight.
  - LayerNorms run as moment sums: the PSUM->SBUF evictions carry
    accum_out partials (sum x), a scalar-engine Square pass accumulates
    sum x^2, and the tiny per-partition algebra runs on vector. LN1 is
    interleaved into pass B's second half so its transposes slot into the
    PE stream while vector/scalar overlap the attention matmuls.
  - FFN1 streams W1 once (group-outer, s-half-inner; last group resident,
    preloaded in phase A). FFN2 runs 3 passes of (3,3,2) s-tiles; W2
    k-tiles 13..15 stay resident so each pass's chains START on them
    (overlapping the previous pass's epilogues) and the per-s-tile chain
    ends stagger on the last streamed tile, keeping the final epilogue
    tail short.
"""

import sys

if "/opt/trn_rl_repo" not in sys.path:
    sys.path.insert(0, "/opt/trn_rl_repo")

import numpy as np
import ml_dtypes

import concourse.bacc as bacc
import concourse.mybir as mybir
import concourse.tile as tile
from concourse.alu_op_type import AluOpType
from concourse.masks import make_identity


AF = mybir.ActivationFunctionType
FP32 = mybir.dt.float32
BF16 = mybir.dt.bfloat16
FP8 = mybir.dt.float8e4
DR = mybir.MatmulPerfMode.DoubleRow

B, S, D, H = 2, 4096, 1024, 4096
N_CORES = 8
G = 4                 # cores per group (one group per batch element)
S_LOC = S // G        # sequence rows per core
P = 128               # SBUF partitions
NF = 512              # matmul moving free-dim (one fp32 PSUM bank)
DT = D // P           # 8 d-tiles
KP = DT // 2          # 4 k-subtile pairs over D
ST = S_LOC // P       # 8 s-tiles per core
TJ = S // P           # 32 global t-subtiles
HT = H // P           # 32 h-tiles
HG = 4                # FFN1 weight-streaming groups
HPG = HT // HG        # 8 h-tiles per group
W2R = 3               # resident W2 tail k-pair tiles (13, 14, 15)
W2S = HT // 2 - W2R   # streamed W2 k-pair tiles per pass (0..12)
EPS = 1e-5
SCL = 32.0            # host pre-scale on Wq/Wk/Wv/W1
SCL2 = 64.0           # host pre-scale on W2
SM_SCALE = 1.0 / float(np.sqrt(np.float32(D)))
EXP_SCALE = SM_SCALE / (SCL * SCL)

GROUPS = [[0, 1, 2, 3], [4, 5, 6, 7]]


def build_graph(nc, tc, ext, trivial_gb):
    stream = ext["stream"]
    persist = ext["persist"]
    stage = ext["stage"]
    const = ext["const"]
    dram = ext["dram"]
    tcx = ext["tc"]

    # ---- constants ----
    ident_bf = const.tile([P, P], BF16, tag="ident_bf", name="ident_bf")
    make_identity(nc, ident_bf[:])
    ident_f = const.tile([P, P], FP32, tag="ident_f", name="ident_f")
    make_identity(nc, ident_f[:])
    ones_dr = const.tile([P, 2, 16], FP8, tag="ones_dr", name="ones_dr")
    nc.vector.memset(ones_dr[:, :, :], 1.0)
    ones_f32 = const.tile([1, P], FP32, tag="ones_f32", name="ones_f32")
    nc.vector.memset(ones_f32[:], 1.0)
    eps_t = const.tile([P, 1], FP32, tag="eps", name="eps")
    nc.vector.memset(eps_t[:], EPS)

    # biases arrive host-packed: [P, 8] bq | [P, 8] bk | [P, 32] b1.
    # (Their DMAs are issued after the first x loads so the cold-start x
    # chunk leads the Sync queue.)
    pvecs = const.tile([P, 48], FP32, tag="pvecs", name="pvecs")
    bq_sb = [pvecs[:, m:m + 1] for m in range(DT)]
    bk_sb = [pvecs[:, DT + m:DT + m + 1] for m in range(DT)]
    b1_sb = [pvecs[:, 2 * DT + m:2 * DT + m + 1] for m in range(HT)]

    # free-dim [1, D] rows at 32-aligned partitions (matmul-legal bases);
    # row 96 is scratch for the softmax recip row (never a matmul operand)
    smalls = const.tile([P, D], FP32, tag="smalls", name="smalls")
    SROW = {"bv": 0, "gamma": 32, "beta": 64}
    smalls2 = const.tile([1, D], FP32, tag="smalls2", name="smalls2")

    def load_small_consts():
        nc.sync.dma_start(out=pvecs[:, 0:DT], in_=ext["bqp_ext"][:, :])
        nc.sync.dma_start(out=pvecs[:, DT:2 * DT], in_=ext["bkp_ext"][:, :])
        nc.sync.dma_start(out=pvecs[:, 2 * DT:2 * DT + HT], in_=ext["b1p_ext"][:, :])
        for nm, r in SROW.items():
            nc.sync.dma_start(out=smalls[r:r + 1, :], in_=ext[nm + "_ext"][0:1, :])
        nc.sync.dma_start(out=smalls2[0:1, :], in_=ext["beta_b2_ext"][0:1, :])

    res = [persist.tile([P, D], FP32, tag=f"res{m}", name=f"res{m}") for m in range(ST)]
    xT_f8 = persist.tile([P, DT, S_LOC], FP8, tag="xT", name="xT")
    qT_f8 = persist.tile([P, DT, S_LOC], FP8, tag="qT", name="qT")
    kT_f8 = persist.tile([P, DT, S_LOC], FP8, tag="kT", name="kT")
    v_half = [persist.tile([P, TJ, NF], FP8, tag=f"vf{h}", name=f"vf{h}")
              for h in range(2)]
    P_f8 = persist.tile([P, TJ, S_LOC], FP8, tag="pf", name="pf")
    # resident W2 tail k-pair tiles (loaded during phase A, used by both
    # FFN2 passes), and FFN1's last weight group (its streamed slot would
    # WAR-stall against group 0's matmuls right at the FFN1->FFN2 seam)
    w2r = [persist.tile([P, 2, D], FP8, tag=f"w2r{i}", name=f"w2r{i}")
           for i in range(W2R)]
    w1g3r = [persist.tile([P, 2, D], FP8, tag=f"w1g3r{i}", name=f"w1g3r{i}")
             for i in range(KP)]

    # gather buffers are partition-major ([p, tile, col] flattened) so the
    # post-gather loads are single DMAs with 4KB-contiguous runs
    ag_q_in = [dram.tile([P, DT * NF], FP8, name=f"agqi{h}") for h in range(2)]
    ag_q_out = [dram.tile([G * P, DT * NF], FP8, name=f"agqo{h}") for h in range(2)]
    ag_v_in = [dram.tile([P, ST * NF], FP8, name=f"agvi{h}") for h in range(2)]
    ag_v_out = [dram.tile([G * P, ST * NF], FP8, name=f"agvo{h}") for h in range(2)]

    bcast = {}
    recipT = const.tile([P, ST], FP32, tag="recipT", name="recipT")

    def load_w8(ext_t, base_row, eng=None):
        eng = eng or nc.sync
        tiles = []
        for kp in range(KP):
            wt = stream.tile([P, 2, D], FP8, tag=f"w{kp}", name=f"w{kp}")
            r0 = base_row + kp * P
            eng.dma_start(out=wt[:, :, :], in_=ext_t[r0:r0 + P, :])
            tiles.append(wt)
        return tiles

    # LayerNorm via moments: sums arrive via accum_out on the evictions
    # (s01), sum-of-squares via a scalar-engine Square pass. Returns
    # (negmu*sd, sd) for the fused apply  (x + nm/sd)*sd = (x-mu)*sd.
    def ln_moments(st, s01):
        lt = stage.tile([P, 8], FP32, tag="lnt", name="lnt", bufs=3)
        sq = stage.tile([P, D], FP32, tag="sqt", name="sqt", bufs=1)
        sumsq = lt[:, 0:1]
        nc.scalar.activation(sq[:], res[st][:], AF.Square, accum_out=sumsq)
        ssum = lt[:, 1:2]
        nc.vector.tensor_add(ssum, s01[:, 0:1], s01[:, 1:2])
        negmu = lt[:, 2:3]
        nc.vector.tensor_scalar_mul(negmu, ssum, -1.0 / D)
        var = lt[:, 3:4]
        mu2 = lt[:, 4:5]
        nc.vector.tensor_mul(mu2, negmu, negmu)
        nc.vector.scalar_tensor_tensor(
            out=var, in0=sumsq, scalar=1.0 / D, in1=mu2,
            op0=AluOpType.mult, op1=AluOpType.subtract,
        )
        sd = lt[:, 5:6]
        nc.scalar.activation(sd, var, AF.Sqrt, bias=eps_t[:])
        nc.vector.reciprocal(sd, sd)
        nm = lt[:, 6:7]
        nc.vector.tensor_mul(nm, negmu, sd)
        return nm, sd

    # evict_eng picks which engine pays for the PSUM->SBUF fp8 eviction
    # (vector during the x loads, scalar inside pass B where vector is the
    # bottleneck)
    def transpose_to(mmp, src_bf, dst_f8, s0, evict_eng=None):
        evict = evict_eng or nc.vector
        tp = mmp.tile([P, DT * P], BF16, tag="trp", name="trp", bufs=2)
        for dj in range(DT):
            nc.tensor.transpose(
                tp[:, dj * P:(dj + 1) * P], src_bf[:, dj * P:(dj + 1) * P],
                ident_bf[:],
            )
        copy_fn = evict.copy if evict is nc.scalar else evict.tensor_copy
        copy_fn(
            out=dst_f8[:, :, s0:s0 + P],
            in_=tp[:].rearrange("p (d s) -> p d s", d=DT),
        )

    # ================= phase A: QKV, attention, LN1, FFN1 =================
    with tcx.tile_pool(name="psA", bufs=1, space="PSUM") as mmp:
        # PE warm-up: ~40 dependency-free matmuls while the first x DMA is
        # in flight. The HAM clock gate needs ~3.4us of sustained PE
        # activity before it lifts the 1.2GHz cold throttle — burning that
        # window on throwaway work makes the real QKV matmuls run at 2.4GHz.
        for _ in range(64):
            wp = mmp.tile([P, NF], FP32, tag="mm", name="mm", bufs=4)
            nc.tensor.matmul(wp[:, 0:P], ident_bf[:], ident_bf[:])
        # ---- x -> xT fp8: first s-half, then q-half0 can go ----
        # loads alternate between the Sync and Scalar queues so the 4MB of
        # x doesn't trickle through a single DMA ring at cold start
        x_eng = [nc.sync, nc.scalar, nc.sync, nc.scalar]

        def load_x_half(h):
            for si in range(h * 4, h * 4 + 4):
                xn = stage.tile([P, D], FP32, tag="stgf", name="stgf", bufs=3)
                xb = stage.tile([P, D], BF16, tag="stgb", name="stgb")
                eng = x_eng[si % 4]
                if si == 0:
                    # split the first tile so the cold-start PE work begins
                    # after ~256KB instead of ~512KB of DMA
                    for c in range(2):
                        eng.dma_start(
                            out=xn[:, c * NF:(c + 1) * NF],
                            in_=ext["x_ext"][si * P:(si + 1) * P, c * NF:(c + 1) * NF],
                        )
                        nc.vector.tensor_copy(
                            out=xb[:, c * NF:(c + 1) * NF],
                            in_=xn[:, c * NF:(c + 1) * NF],
                        )
                else:
                    eng.dma_start(out=xn[:], in_=ext["x_ext"][si * P:(si + 1) * P, :])
                    nc.vector.tensor_copy(out=xb[:], in_=xn[:])
                transpose_to(mmp, xb, xT_f8, si * P)

        def q_half(h):
            n0 = h * NF
            for m in range(DT):
                pt = mmp.tile([P, NF], FP32, tag="mm", name="mm", bufs=4)
                for kp in range(KP):
                    nc.tensor.matmul(
                        pt[:], wq[kp][:, :, m * P:(m + 1) * P],
                        xT_f8[:, 2 * kp:2 * kp + 2, n0:n0 + NF],
                        start=(kp == 0), stop=(kp == KP - 1), perf_mode=DR,
                    )
                nc.scalar.activation(qT_f8[:, m, n0:n0 + NF], pt[:], AF.Identity,
                                     bias=bq_sb[m])
                nc.gpsimd.dma_start(
                    out=ag_q_in[h][:, m * NF:(m + 1) * NF],
                    in_=qT_f8[:, m, n0:n0 + NF],
                )
            nc.gpsimd.collective_compute(
                "AllGather", AluOpType.bypass, replica_groups=GROUPS,
                ins=[ag_q_in[h][:].opt()], outs=[ag_q_out[h][:].opt()],
            )

        load_x_half(0)
        wq = load_w8(ext["wq8_ext"], 0, eng=nc.scalar)
        load_small_consts()
        q_half(0)
        load_x_half(1)
        q_half(1)

        # ---- v = x @ (32 Wv) + 32 bv (natural, fp8); AllGather ----
        wv = load_w8(ext["wv8_ext"], 0)
        bv_b = const.tile([P, D], FP32, tag="bc_bv", name="bc_bv")
        for n0 in range(0, D, NF):
            pt = mmp.tile([P, NF], FP32, tag="mm", name="mm", bufs=4)
            nc.tensor.matmul(pt[:], ones_f32[0:1, :], smalls[0:1, n0:n0 + NF])
            nc.scalar.copy(out=bv_b[:, n0:n0 + NF], in_=pt[:])
        for mt in range(ST):
            v8 = stage.tile([P, D], FP8, tag="v8", name="v8")
            for n0 in range(0, D, NF):
                pt = mmp.tile([P, NF], FP32, tag="mm", name="mm", bufs=4)
                for kp in range(KP):
                    nc.tensor.matmul(
                        pt[:], xT_f8[:, 2 * kp:2 * kp + 2, mt * P:(mt + 1) * P],
                        wv[kp][:, :, n0:n0 + NF],
                        start=(kp == 0), stop=(kp == KP - 1), perf_mode=DR,
                    )
                nc.vector.tensor_add(
                    v8[:, n0:n0 + NF], pt[:], bv_b[:, n0:n0 + NF]
                )
            for hh in range(2):
                nc.gpsimd.dma_start(
                    out=ag_v_in[hh][:, mt * NF:(mt + 1) * NF],
                    in_=v8[:, hh * NF:(hh + 1) * NF],
                )
        # v gathered in two d-halves so pass B's first half can start while
        # the second half is still on the wire (the CC stream is serial)
        for hh in range(2):
            nc.gpsimd.collective_compute(
                "AllGather", AluOpType.bypass, replica_groups=GROUPS,
                ins=[ag_v_in[hh][:].opt()], outs=[ag_v_out[hh][:].opt()],
            )

        # ---- kT = (32 Wk).T @ x + 32 bk (fp8, local) ----
        wk = load_w8(ext["wk8_ext"], 0)
        # resident W2 tail tiles + FFN1's last weight group: load now on the
        # Scalar queue (no sem waits there) so the Sync queue's qch stream
        # leads right at the q0-mesh end (consumed in FFN1/FFN2)
        for i in range(W2R):
            kp2 = W2S + i
            nc.scalar.dma_start(
                out=w2r[i][:, :, :], in_=ext["w28_ext"][kp2 * P:(kp2 + 1) * P, :]
            )
        for kp in range(KP):
            r0 = ((HG - 1) * KP + kp) * P
            nc.scalar.dma_start(
                out=w1g3r[kp][:, :, :], in_=ext["w18_ext"][r0:r0 + P, :]
            )
        for m in range(DT):
            for n0 in range(0, S_LOC, NF):
                pt = mmp.tile([P, NF], FP32, tag="mm", name="mm", bufs=4)
                for kp in range(KP):
                    nc.tensor.matmul(
                        pt[:], wk[kp][:, :, m * P:(m + 1) * P],
                        xT_f8[:, 2 * kp:2 * kp + 2, n0:n0 + NF],
                        start=(kp == 0), stop=(kp == KP - 1), perf_mode=DR,
                    )
                nc.scalar.activation(kT_f8[:, m, n0:n0 + NF], pt[:], AF.Identity,
                                     bias=bk_sb[m])

        # [P, D] broadcasts, off the critical path (fills AG wait)
        bc_rows = [("gamma", smalls[32:33, :]),
                   ("beta", smalls[64:65, :]),
                   ("beta_b2", smalls2[0:1, :])]
        if trivial_gb:
            bc_rows = [bc_rows[2]]  # only beta+b2 needed
        for nm, srow in bc_rows:
            bt = const.tile([P, D], FP32, tag=f"bc_{nm}", name=f"bc_{nm}")
            for n0 in range(0, D, NF):
                pt = mmp.tile([P, NF], FP32, tag="mm", name="mm", bufs=4)
                nc.tensor.matmul(pt[:], ones_f32[0:1, :], srow[:, n0:n0 + NF])
                nc.scalar.copy(out=bt[:, n0:n0 + NF], in_=pt[:])
            bcast[nm] = bt

        # ---- pass A: P[t, s] = exp(k·q/sqrt(D)); DR rowsums 1 chunk back ----
        rs_ps = [mmp.tile([1, NF], FP32, tag=f"rs{h}", name=f"rs{h}", bufs=1)
                 for h in range(2)]
        chunks = [(ht, r) for ht in range(2) for r in range(G)]

        def emit_rowsum(ci):
            ht, r = chunks[ci]
            jp0 = (r * ST + ht * 4) // 2
            for h in range(2):
                n0 = h * NF
                for jj in range(2):
                    a = 2 * ci + jj
                    nc.tensor.matmul(
                        rs_ps[h][:], ones_dr[:, :, 0:1],
                        P_f8[:, 2 * (jp0 + jj):2 * (jp0 + jj) + 2, n0:n0 + NF],
                        start=(a == 0), stop=(a == 2 * len(chunks) - 1),
                        perf_mode=DR,
                    )

        qtiles = {}

        def issue_qch(ci):
            ht, r = chunks[ci]
            qch = stream.tile([P, DT, NF], FP8, tag="q", name="q", bufs=3)
            nc.sync.dma_start(
                out=qch[:, :, :], in_=ag_q_out[ht][r * P:(r + 1) * P, :]
            )
            qtiles[ci] = qch

        issue_qch(0)
        issue_qch(1)
        for ci, (ht, r) in enumerate(chunks):
            if ci + 2 < len(chunks):
                issue_qch(ci + 2)
            qch = qtiles.pop(ci)
            for tti in range(4):
                j = r * ST + ht * 4 + tti
                for n0 in range(0, S_LOC, NF):
                    ps = mmp.tile([P, NF], FP32, tag="mm", name="mm", bufs=4)
                    for kp in range(KP):
                        nc.tensor.matmul(
                            ps[:], qch[:, 2 * kp:2 * kp + 2, tti * P:(tti + 1) * P],
                            kT_f8[:, 2 * kp:2 * kp + 2, n0:n0 + NF],
                            start=(kp == 0), stop=(kp == KP - 1), perf_mode=DR,
                        )
                    nc.scalar.activation(
                        P_f8[:, j, n0:n0 + NF], ps[:], AF.Exp, scale=EXP_SCALE
                    )
            if ci > 0:
                emit_rowsum(ci - 1)
        emit_rowsum(len(chunks) - 1)

        # raw rowsums -> smalls row 96 (scalar copies: a [1,512] vector op
        # is single-partition and takes ~3.3us); the reciprocal runs after
        # the transpose in [P, 8] form where it's ~100x faster. The tiny
        # transpose is emitted inside pass B (after st0's matmuls) so the
        # PE queue doesn't stall on it before the attention matmuls.
        rs_row = smalls[96:97, :]
        for h in range(2):
            nc.scalar.copy(out=rs_row[0:1, h * NF:(h + 1) * NF], in_=rs_ps[h][:])
        rs8 = const.tile([ST, P], FP32, tag="rs8", name="rs8")
        nc.scalar.dma_start(out=rs8[:, :], in_=rs_row[0:1, :])

        # ---- pass B: attn natural [s, d] + residual -> res (fp32) ----
        # d-half outer: half 0 computes while v's half-1 gather is in flight.
        # gathered-v loads ride the GpSimd queue (idle after the triggers) so
        # the Sync queue's qch stream can't delay them; they're split per
        # peer r-block so the jp accumulation chain (which consumes r-blocks
        # in order) can start on the first block when the mesh runs late.
        for hh in range(2):
            for r in range(G):
                nc.gpsimd.dma_start(
                    out=v_half[hh][:, r * ST:(r + 1) * ST, :].rearrange(
                        "p m c -> p (m c)"),
                    in_=ag_v_out[hh][r * P:(r + 1) * P, :],
                )

        # LN1 interleaved into the h=1 evictions: stats/apply/cast overlap
        # the next s-tile's attention matmuls; transposes slot into the PE
        # stream.
        x1T_f8 = persist.tile([P, DT, S_LOC], FP8, tag="xT", name="xT")
        h_sh = [persist.tile([P, TJ, NF], FP8, tag=f"vf{h}", name=f"vf{h}")
                for h in range(2)]

        s01 = {}
        for h in range(2):
            n0 = h * NF
            for st in range(ST):
                xre = stage.tile([P, NF], FP32, tag="xre", name="xre")
                nc.scalar.dma_start(
                    out=xre[:], in_=ext["x_ext"][st * P:(st + 1) * P, n0:n0 + NF]
                )
                ps = mmp.tile([P, NF], FP32, tag="mm", name="mm", bufs=4)
                for jp in range(TJ // 2):
                    nc.tensor.matmul(
                        ps[:], P_f8[:, 2 * jp:2 * jp + 2, st * P:(st + 1) * P],
                        v_half[h][:, 2 * jp:2 * jp + 2, :],
                        start=(jp == 0), stop=(jp == TJ // 2 - 1), perf_mode=DR,
                    )
                if h == 0 and st == 0:
                    rt_ps = mmp.tile([P, NF], FP32, tag="mm", name="mm", bufs=4)
                    nc.tensor.transpose(rt_ps[:, 0:ST], rs8[:, :],
                                        ident_f[0:ST, 0:ST])
                    nc.scalar.activation(recipT[:], rt_ps[:, 0:ST], AF.Identity,
                                         scale=SCL)
                    nc.vector.reciprocal(recipT[:], recipT[:])
                if h == 0:
                    sx = stage.tile([P, 2], FP32, tag="s01", name="s01", bufs=8)
                    s01[st] = sx
                    acc = sx[:, 0:1]
                else:
                    acc = s01[st][:, 1:2]
                nc.vector.scalar_tensor_tensor(
                    out=res[st][:, n0:n0 + NF], in0=ps[:], scalar=recipT[:, st:st + 1],
                    in1=xre[:], op0=AluOpType.mult, op1=AluOpType.add,
                    accum_out=acc,
                )
                if h == 1:
                    # LN1: res[st] <- z (normalized); then cast+transpose
                    nm, sd = ln_moments(st, s01[st])
                    nc.vector.tensor_scalar(
                        res[st][:], res[st][:], nm, sd,
                        op0=AluOpType.add, op1=AluOpType.mult,
                    )
                    xb = stage.tile([P, D], BF16, tag="stgb", name="stgb")
                    nc.vector.tensor_copy(out=xb[:], in_=res[st][:])
                    transpose_to(mmp, xb, x1T_f8, st * P, evict_eng=nc.scalar)

        # ---- FFN1: stream W1 once (group-outer, s-half-inner) ----
        for g in range(HG):
            w1g = (w1g3r if g == HG - 1
                   else load_w8(ext["w18_ext"], g * KP * P))
            for sh in range(2):
                n0 = sh * NF
                for mh_i in range(HPG):
                    mh = g * HPG + mh_i
                    pt = mmp.tile([P, NF], FP32, tag="mm", name="mm", bufs=4)
                    for kp in range(KP):
                        nc.tensor.matmul(
                            pt[:], w1g[kp][:, :, mh_i * P:(mh_i + 1) * P],
                            x1T_f8[:, 2 * kp:2 * kp + 2, n0:n0 + NF],
                            start=(kp == 0), stop=(kp == KP - 1), perf_mode=DR,
                        )
                    nc.scalar.activation(
                        h_sh[sh][:, mh, :], pt[:], AF.Gelu,
                        bias=b1_sb[mh], scale=1.0 / SCL,
                    )

    # ================= phase B: FFN2 (fp8 DR) + LN2 + out =================
    # 3 passes of (3, 3, 2) s-tiles. Chains START on the resident W2 tail
    # tiles (which also overlap the previous pass's epilogues), then
    # consume streamed k 0..12; per-s-tile chain ends stagger on k=12 so
    # the epilogues pipeline, and the 2-tile last pass keeps the final
    # epilogue tail short.
    with tcx.tile_pool(name="psB", bufs=1, space="PSUM") as f2p:
        for sp, sts in enumerate([(0, 1, 2), (3, 4, 5), (6, 7)]):
            f2 = {(st, h): f2p.tile([P, NF], FP32, tag=f"f{i}_{h}",
                                    name=f"f{i}_{h}")
                  for i, st in enumerate(sts) for h in range(2)}

            def f2mm(kp2, st, h, w2t, start=False, stop=False):
                nc.tensor.matmul(
                    f2[(st, h)][:],
                    h_sh[st // 4][:, 2 * kp2:2 * kp2 + 2,
                                  (st % 4) * P:(st % 4 + 1) * P],
                    w2t[:, :, h * NF:(h + 1) * NF],
                    start=start, stop=stop, perf_mode=DR,
                )

            # resident tail first: starts the accumulation groups
            for st in sts:
                for h in range(2):
                    for i in range(W2R):
                        f2mm(W2S + i, st, h, w2r[i], start=(i == 0))

            # streamed k-pairs 0..W2S-1; stagger chain ends on the last one
            for kp2 in range(W2S):
                wt = stream.tile([P, 2, D], FP8, tag=f"w{kp2 % KP}",
                                 name=f"w{kp2 % KP}")
                nc.sync.dma_start(
                    out=wt[:, :, :],
                    in_=ext["w28_ext"][kp2 * P:(kp2 + 1) * P, :],
                )
                if kp2 == 0 and trivial_gb:
                    # pre-add beta+b2 into res here (vector is idle during
                    # the matmul stream) so the per-tile tail epilogue shrinks
                    for st in sts:
                        nc.vector.tensor_add(res[st][:], res[st][:],
                                             bcast["beta_b2"][:])
                if kp2 < W2S - 1:
                    for st in sts:
                        for h in range(2):
                            f2mm(kp2, st, h, wt)
                else:
                    for st in sts:
                        for h in range(2):
                            f2mm(kp2, st, h, wt, stop=True)
                        # epilogue streams per s-tile as its chains end
                        s2 = stage.tile([P, 2], FP32, tag="s01", name="s01",
                                        bufs=8)
                        if trivial_gb:
                            # pre-LN2 = z + (beta+b2) + f2/SCL2 (beta_b2
                            # pre-added above)
                            for h in range(2):
                                n0 = h * NF
                                nc.vector.scalar_tensor_tensor(
                                    out=res[st][:, n0:n0 + NF], in0=f2[(st, h)][:],
                                    scalar=1.0 / SCL2, in1=res[st][:, n0:n0 + NF],
                                    op0=AluOpType.mult, op1=AluOpType.add,
                                    accum_out=s2[:, h:h + 1],
                                )
                        else:
                            t2 = stage.tile([P, D], FP32, tag="stgf2", name="stgf2")
                            nc.vector.tensor_mul(t2[:], res[st][:], bcast["gamma"][:])
                            for h in range(2):
                                n0 = h * NF
                                nc.vector.scalar_tensor_tensor(
                                    out=t2[:, n0:n0 + NF], in0=f2[(st, h)][:],
                                    scalar=1.0 / SCL2, in1=t2[:, n0:n0 + NF],
                                    op0=AluOpType.mult, op1=AluOpType.add,
                                )
                            nc.vector.tensor_add(res[st][:], t2[:],
                                                 bcast["beta_b2"][:])
                            nc.vector.reduce_sum(s2[:, 0:1], res[st][:, 0:NF])
                            nc.vector.reduce_sum(s2[:, 1:2], res[st][:, NF:D])
                        # LN2 + store; the apply runs on scalar (vector is
                        # the epilogue bottleneck), per d-half so the final
                        # out DMA starts half a tile earlier
                        nm, sd = ln_moments(st, s2)
                        ot = stage.tile([P, D], FP32, tag="stgf", name="stgf",
                                        bufs=3)
                        for h in range(2):
                            n0 = h * NF
                            nc.scalar.activation(
                                ot[:, n0:n0 + NF], res[st][:, n0:n0 + NF],
                                AF.Identity, bias=nm, scale=sd,
                            )
                            if not trivial_gb:
                                nc.vector.tensor_mul(ot[:, n0:n0 + NF],
                                                     ot[:, n0:n0 + NF],
                                                     bcast["gamma"][:, n0:n0 + NF])
                                nc.vector.tensor_add(ot[:, n0:n0 + NF],
                                                     ot[:, n0:n0 + NF],
                                                     bcast["beta"][:, n0:n0 + NF])
                            nc.gpsimd.dma_start(
                                out=ext["out_ext"][st * P:(st + 1) * P,
                                                   n0:n0 + NF],
                                in_=ot[:, n0:n0 + NF],
                            )


def build_nc(trivial_gb):
    nc = bacc.Bacc(target_bir_lowering=False, num_devices=N_CORES)

    ext = {
        "x_ext": nc.declare_dram_parameter("x", [S_LOC, D], FP32, isOutput=False),
        "wq8_ext": nc.declare_dram_parameter("wq8", [KP * P, 2 * D], FP8, isOutput=False),
        "wk8_ext": nc.declare_dram_parameter("wk8", [KP * P, 2 * D], FP8, isOutput=False),
        "wv8_ext": nc.declare_dram_parameter("wv8", [KP * P, 2 * D], FP8, isOutput=False),
        "w18_ext": nc.declare_dram_parameter("w18", [HG * KP * P, 2 * D], FP8, isOutput=False),
        "w28_ext": nc.declare_dram_parameter("w28", [(HT // 2) * P, 2 * D], FP8, isOutput=False),
        "bqp_ext": nc.declare_dram_parameter("bqp", [P, DT], FP32, isOutput=False),
        "bkp_ext": nc.declare_dram_parameter("bkp", [P, DT], FP32, isOutput=False),
        "b1p_ext": nc.declare_dram_parameter("b1p", [P, HT], FP32, isOutput=False),
        "bv_ext": nc.declare_dram_parameter("bv", [1, D], FP32, isOutput=False),
        "beta_b2_ext": nc.declare_dram_parameter("beta_b2", [1, D], FP32, isOutput=False),
        "gamma_ext": nc.declare_dram_parameter("gamma", [1, D], FP32, isOutput=False),
        "beta_ext": nc.declare_dram_parameter("beta", [1, D], FP32, isOutput=False),
        "out_ext": nc.declare_dram_parameter("out", [S_LOC, D], FP32, isOutput=True),
    }

    with tile.TileContext(nc) as tc:
        with (
            tc.tile_pool(name="dram", bufs=1, space="DRAM") as dram,
            tc.tile_pool(name="const", bufs=1) as const,
            tc.tile_pool(name="persist", bufs=1) as persist,
            tc.tile_pool(name="stage", bufs=2) as stage,
            tc.tile_pool(name="stream", bufs=2) as stream,
        ):
            ext.update(tc=tc, dram=dram, const=const, persist=persist,
                       stage=stage, stream=stream)
            build_graph(nc, tc, ext, trivial_gb)
    nc.compile()
    return nc


_NC_CACHE = {}


def _get_nc(trivial_gb):
    if trivial_gb not in _NC_CACHE:
        _NC_CACHE[trivial_gb] = build_nc(trivial_gb)
    return _NC_CACHE[trivial_gb]


F8NP = ml_dtypes.float8_e4m3


def _pair_rows(w):
    # [K, N] -> pair layout: rows kp*128+p, cols i*N+c = w[(2kp+i)*128+p, c]
    k, n = w.shape
    kp = k // (2 * P)
    w4 = w.reshape(kp, 2, P, n).transpose(0, 2, 1, 3).reshape(kp * P, 2 * n)
    return np.ascontiguousarray(w4)


def _col_pack(v, n):
    # [n*128] -> [128, n] with out[p, m] = v[m*128 + p]
    return np.ascontiguousarray(v.reshape(n, P).T)


def _make_in_maps(inputs):
    x = np.asarray(inputs["input_embedding"], dtype=np.float32)
    assert x.shape == (B, S, D), x.shape

    gamma = np.asarray(inputs["gamma"], np.float32).reshape(D)
    beta = np.asarray(inputs["beta"], np.float32).reshape(D)
    trivial_gb = bool(np.all(gamma == 1.0) and np.all(beta == 0.0))
    W1 = np.asarray(inputs["W1"], np.float32)
    b1 = np.asarray(inputs["b1"], np.float32).reshape(H)
    # fold LN1's gamma/beta into W1/b1 (FFN1 consumes the normalized z)
    W1f = gamma[:, None] * W1
    b1f = b1 + beta @ W1
    # W1 group-major pair layout: rows (g*KP+kp)*128+p, cols i*D+c
    w1g = (SCL * W1f).reshape(KP, 2, P, HG, D).transpose(3, 0, 2, 1, 4)
    w18 = np.ascontiguousarray(w1g.reshape(HG * KP * P, 2 * D)).astype(F8NP)

    shared = {
        "wq8": _pair_rows(SCL * np.asarray(inputs["Wq"], np.float32)).astype(F8NP),
        "wk8": _pair_rows(SCL * np.asarray(inputs["Wk"], np.float32)).astype(F8NP),
        "wv8": _pair_rows(SCL * np.asarray(inputs["Wv"], np.float32)).astype(F8NP),
        "w18": w18,
        "w28": _pair_rows(SCL2 * np.asarray(inputs["W2"], np.float32)).astype(F8NP),
        "bqp": _col_pack(SCL * np.asarray(inputs["bq"], np.float32).reshape(D), DT),
        "bkp": _col_pack(SCL * np.asarray(inputs["bk"], np.float32).reshape(D), DT),
        "b1p": _col_pack(b1f, HT),
        "bv": SCL * np.asarray(inputs["bv"], np.float32).reshape(1, D),
        "beta_b2": (beta + np.asarray(inputs["b2"], np.float32).reshape(D)).reshape(1, D),
        "gamma": gamma.reshape(1, D),
        "beta": beta.reshape(1, D),
    }

    in_maps = []
    for c in range(N_CORES):
        b = c // G
        r = c % G
        m = dict(shared)
        m["x"] = np.ascontiguousarray(x[b, r * S_LOC:(r + 1) * S_LOC, :])
        in_maps.append(m)
    return in_maps, trivial_gb


def kernel(**inputs: np.ndarray) -> np.ndarray:
    from concourse.bass_utils import run_bass_kernel_spmd

    in_maps, trivial_gb = _make_in_maps(inputs)
    nc = _get_nc(trivial_gb)
    res = run_bass_kernel_spmd(nc, in_maps, core_ids=list(range(N_CORES)))

    out = np.empty((B, S, D), dtype=np.float32)
    for c in range(N_CORES):
        b = c // G
        r = c % G
        out[b, r * S_LOC:(r + 1) * S_LOC, :] = res.results[c]["out"]
    return out
